# revision 29
# baseline (speedup 1.0000x reference)
"""GIN-style 3-layer GNN encoder on 8 Trainium2 NeuronCores (Bass/Tile).

Reference computation (fp32):
    h = x @ W_in.T + b_in                                  [50000, 96]
    for l in 0..2:
        agg = segment_sum(h[src], dst, N)                  [50000, 96]
        h = (h + agg) @ W_layers[l].T + b_layers[l]
    out = concat([h0..h3], 1) @ W_out.T + b_out            [50000, 128]

Distribution: nodes are partitioned across the 8 cores (6250/core) via a
host-side balancing permutation; each edge is owned by the core that owns
its dst node.  Each layer the updated node features are AllGathered into
two replicated row-major fp16 tables h_fullA/h_fullB (first/second half
of every core's node range, 25000 x 256B rows each, one Shared-space
pair per state so collectives write once and never alias) — the split
halves the AllGather latency on the critical path and keeps gather
indices < 32768 (int16).

Per-core segment sum: a core's node range is split into 49 windows of 128
nodes.  Every window has a fixed number of 128-edge tiles (T_a tiles with
src in half A, T_b in half B; the balancing permutation equalizes
per-window per-class edge counts so the fixed tile counts are tight).
Edge features are fetched with gpsimd dma_gather (fp16 256B rows, 1024
idxs per instruction, round-robin over the 4 SWDGE queues).  For each
window the one-hot onehot[e, j, t] = (j == dst_local[e, t]) is built on
DVE (layout [128, WIN, T] keeps every operand's last dim stride-1 so the
2x DVE perf mode engages), and the PE accumulates
    psum[96, 128] += gathered_tile[128e, 96].T @ onehot[:, :, t]
which is aggT for the window.

The whole layer is chunk-pipelined: each chunk of 4 windows (512 nodes =
one PSUM bank) flows gathers -> onehot/agg -> +h (DVE) -> layer matmul ->
bias -> PE transpose to the row-major fp16 shard.  bounceA DMAs fire
mid-layer (after window 24) and AllGather-A for the next table is emitted
mid-gather-stream (chunk 9) so its flight overlaps the rest of the layer;
AllGather-B fires at layer end (a collective's sequencer wait only blocks
until its input is ready — the flight itself runs async on the CC cores).
The final output projection is interleaved into layer 2's chunk loop.
"""
import sys

sys.path.insert(0, "/opt/trn_rl_repo")

import numpy as np

N_NODES = 50000
N_EDGES = 800000
IN_DIM = 128
HID = 96
OUT_DIM = 128
N_LAYERS = 3
N_CORES = 8
NPC = N_NODES // N_CORES          # 6250 nodes per core
WIN = 128                         # window width (nodes)
NW = (NPC + WIN - 1) // WIN       # 49 windows per core (last = 106 nodes)
HALF = 4096                       # per-core A/B split: A = first 4096 nodes
                                  # (global A = 32768 rows = int16 limit);
                                  # asymmetric so AllGather-B is small and
                                  # lands quickly at the layer boundary
BHALF = NPC - HALF                # 2154 B nodes per core
CLS = N_CORES * HALF              # 32768: A-class size
AW = HALF // WIN                  # 32 full-A windows per core
REM_A = HALF - AW * WIN           # 0 A-slots in window AW
CHUNK_W = 4                       # windows per chunk (= 512 nodes = 1 bank)
AGW = 64                          # aggregation window width (PE onehot
                                  # matmul free dim: 64 halves the MM
                                  # stream cost vs 128 at ~+6% tile pad)
NWA = (NPC + AGW - 1) // AGW      # 98 agg windows per core (last = 42)
APC = CHUNK_W * WIN // AGW        # 8 agg windows per chunk
GT = 8                            # tiles per dma_gather (1024 idxs = the
                                  # 64-descs-per-engine single_packet cap)
SINGLE_PACKET = True
CW_N = CHUNK_W * WIN              # 512: node-chunk for dense matmuls
BOUNCE_A_CHUNK = (AW * WIN + REM_A - 1) // CW_N   # chunk whose transposes
                                                  # complete the A half (6)

_cache = {}


def _balance_nodes(src0, dst0):
    """Permute node ids so per-(core,window) A/B edge counts are even.

    A node's A/B class (which replicated gather table its row lives in) is
    frozen to its OLD id (< CLS -> A); the permutation only moves nodes
    within their class region, so per-node (deg_a, deg_b) are fixed and a
    greedy 2-D balance over the 392 (core, window) bins makes the uniform
    tile counts T_a/T_b tight.  Returns perm (old id -> new id).
    """
    deg_a = np.bincount(dst0[src0 < CLS], minlength=N_NODES).astype(np.int64)
    deg_b = np.bincount(dst0[src0 >= CLS], minlength=N_NODES).astype(np.int64)
    nbins = N_CORES * NWA
    base = np.empty(nbins, np.int64)
    cap = np.empty(nbins, np.int64)
    for b in range(nbins):
        c, w = divmod(b, NWA)
        base[b] = c * NPC + w * AGW
        cap[b] = min(AGW, NPC - w * AGW)
    woff = base % NPC
    q_a = np.clip(HALF - woff, 0, cap)   # A slots = first q_a of the window
    q_b = cap - q_a

    mu_a = max(1.0, deg_a.sum() / nbins)
    mu_b = max(1.0, deg_b.sum() / nbins)
    order = np.argsort(-(deg_a + deg_b), kind="stable")
    a_load = np.zeros(nbins)
    b_load = np.zeros(nbins)
    a_left = q_a.copy()
    b_left = q_b.copy()
    a_pos = np.zeros(nbins, np.int64)
    b_pos = q_a.copy()
    perm = np.empty(N_NODES, np.int64)
    for n in order:
        phi = np.maximum((a_load + deg_a[n]) / mu_a,
                         (b_load + deg_b[n]) / mu_b)
        if n < CLS:
            phi = np.where(a_left > 0, phi, np.inf)
            b_ = int(np.argmin(phi))
            perm[n] = base[b_] + a_pos[b_]
            a_pos[b_] += 1
            a_left[b_] -= 1
        else:
            phi = np.where(b_left > 0, phi, np.inf)
            b_ = int(np.argmin(phi))
            perm[n] = base[b_] + b_pos[b_]
            b_pos[b_] += 1
            b_left[b_] -= 1
        a_load[b_] += deg_a[n]
        b_load[b_] += deg_b[n]
    return perm


def _prep(edge_index):
    """Host-side edge bucketing -> per-core gather index / dst tables."""
    src0 = edge_index[0].astype(np.int64)
    dst0 = edge_index[1].astype(np.int64)
    perm = _balance_nodes(src0, dst0)
    src = perm[src0]
    dst = perm[dst0]
    core = dst // NPC
    din = dst % NPC
    w = din // AGW
    dstl = din % AGW
    s_in = src % NPC
    c_src = src // NPC
    is_b = (s_in >= HALF).astype(np.int64)
    pos = np.where(is_b == 0, c_src * HALF + s_in,
                   c_src * BHALF + s_in - HALF)  # < 32768, int16-safe

    key = (core * NWA + w) * 2 + is_b
    order = np.argsort(key, kind="stable")
    s_pos = pos[order]
    s_dstl = dstl[order]
    s_key = key[order]
    s_b = is_b[order]

    counts = np.bincount(key, minlength=N_CORES * NWA * 2)
    T_a = max(1, int(-(-counts.reshape(-1, 2)[:, 0].max() // 128)))
    T_b = max(1, int(-(-counts.reshape(-1, 2)[:, 1].max() // 128)))
    T = T_a + T_b

    starts = np.zeros(N_CORES * NWA * 2, np.int64)
    starts[1:] = np.cumsum(counts)[:-1]
    rank = np.arange(len(s_key)) - starts[s_key]

    c_arr = s_key // (2 * NWA)
    w_arr = (s_key // 2) % NWA

    idx_a = np.zeros((N_CORES, NWA, T_a * 128), np.int16)
    idx_b = np.zeros((N_CORES, NWA, T_b * 128), np.int16)
    dstl_arr = np.full((N_CORES, NWA, T, 128), -1.0, np.float16)

    a_m = s_b == 0
    flat = (c_arr[a_m] * NWA + w_arr[a_m]) * (T_a * 128) + rank[a_m]
    idx_a.reshape(-1)[flat] = s_pos[a_m].astype(np.int16)
    t_g = rank[a_m] // 128
    e_g = rank[a_m] % 128
    flat = ((c_arr[a_m] * NWA + w_arr[a_m]) * T + t_g) * 128 + e_g
    dstl_arr.reshape(-1)[flat] = s_dstl[a_m].astype(np.float16)

    b_m = ~a_m
    flat = (c_arr[b_m] * NWA + w_arr[b_m]) * (T_b * 128) + rank[b_m]
    idx_b.reshape(-1)[flat] = s_pos[b_m].astype(np.int16)
    t_g = rank[b_m] // 128 + T_a
    e_g = rank[b_m] % 128
    flat = ((c_arr[b_m] * NWA + w_arr[b_m]) * T + t_g) * 128 + e_g
    dstl_arr.reshape(-1)[flat] = s_dstl[b_m].astype(np.float16)

    def wrap(vals):  # [NW*Tc*128] -> [128, NW*Tc*8] int16 wrapped+replicated
        v = vals.reshape(-1, 16).T
        return np.tile(v, (8, 1)).copy()

    idx_a_w = np.stack([wrap(idx_a[c].reshape(-1)) for c in range(N_CORES)])
    idx_b_w = np.stack([wrap(idx_b[c].reshape(-1)) for c in range(N_CORES)])
    dstloc = np.ascontiguousarray(dstl_arr.transpose(0, 3, 1, 2))  # [C,128,NW,T]
    return idx_a_w, idx_b_w, dstloc, T_a, T_b, perm


def _build(T_a, T_b):
    from concourse import bacc, tile, mybir, library_config

    dt = mybir.dt
    T = T_a + T_b
    nc = bacc.Bacc("TRN2", target_bir_lowering=False, debug=False,
                   num_devices=N_CORES, num_swdge_queues=4,
                   dynamic_dma_scratch_size=49152)

    # ---- I/O ----
    xT_in = nc.dram_tensor("xT", [IN_DIM, NPC], dt.float32, kind="ExternalInput")
    w_inT_in = nc.dram_tensor("w_inT", [IN_DIM, HID], dt.float32,
                              kind="ExternalInput")
    b_in_in = nc.dram_tensor("b_in", [HID, 1], dt.float32, kind="ExternalInput")
    w_lT_in = nc.dram_tensor("w_lT", [N_LAYERS, HID, HID], dt.float32,
                             kind="ExternalInput")
    b_l_in = nc.dram_tensor("b_l", [N_LAYERS, HID, 1], dt.float32,
                            kind="ExternalInput")
    w_out4_in = nc.dram_tensor("w_out4", [N_LAYERS + 1, HID, OUT_DIM],
                               dt.float16, kind="ExternalInput")
    b_out_in = nc.dram_tensor("b_out", [OUT_DIM, 1], dt.float32,
                              kind="ExternalInput")
    iota_in = nc.dram_tensor("iota_wt", [128, AGW, T], dt.float16,
                             kind="ExternalInput")
    id96_in = nc.dram_tensor("id96", [HID, HID], dt.float16,
                             kind="ExternalInput")
    id128_in = nc.dram_tensor("id128", [128, 128], dt.float32,
                              kind="ExternalInput")
    idx_a_in = nc.dram_tensor("idx_a", [128, NWA * T_a * 8], dt.int16,
                              kind="ExternalInput")
    idx_b_in = nc.dram_tensor("idx_b", [128, NWA * T_b * 8], dt.int16,
                              kind="ExternalInput")
    dstloc_in = nc.dram_tensor("dstloc", [128, NWA, T], dt.float16,
                               kind="ExternalInput")
    out_ext = nc.dram_tensor("out", [NPC, OUT_DIM], dt.float32,
                             kind="ExternalOutput")

    f32, f32r, f16 = dt.float32, dt.float32r, dt.float16

    with tile.TileContext(nc, num_cores=N_CORES) as tc:
        nc.gpsimd.load_library(library_config.mlp)
        with tc.tile_pool(name="persist", bufs=1) as pp, \
             tc.tile_pool(name="xpool", bufs=2) as xpool, \
             tc.tile_pool(name="hp", bufs=3) as hp_pool, \
             tc.tile_pool(name="ga", bufs=3) as ga_pool, \
             tc.tile_pool(name="gb", bufs=3) as gb_pool, \
             tc.tile_pool(name="oha", bufs=6) as oha_pool, \
             tc.tile_pool(name="ohb", bufs=6) as ohb_pool, \
             tc.tile_pool(name="otile", bufs=2) as ot_pool, \
             tc.tile_pool(name="ps_agg", bufs=3, space="PSUM") as ps_agg, \
             tc.tile_pool(name="ps_big", bufs=2, space="PSUM") as ps_big, \
             tc.tile_pool(name="ps_tr", bufs=2, space="PSUM") as ps_tr, \
             tc.tile_pool(name="dram", bufs=1, space="DRAM") as dram:

            def load(name, shape, dtype, src_ap):
                t = pp.tile(shape, dtype, name=name)
                nc.sync.dma_start(out=t[:], in_=src_ap)
                return t

            w_inT = load("w_inT", [IN_DIM, HID], f32r, w_inT_in[:].bitcast(f32r))
            b_in = load("b_in", [HID, 1], f32, b_in_in[:])
            w_lT = [load(f"w_lT{l}", [HID, HID], f32r, w_lT_in[l].bitcast(f32r))
                    for l in range(N_LAYERS)]
            b_l = [load(f"b_l{l}", [HID, 1], f32, b_l_in[l])
                   for l in range(N_LAYERS)]
            w_out4 = [load(f"w_out4_{s}", [HID, OUT_DIM], f16, w_out4_in[s])
                      for s in range(N_LAYERS + 1)]
            b_out = load("b_out", [OUT_DIM, 1], f32, b_out_in[:])
            iota_wt = load("iota_wt", [128, AGW, T], f16, iota_in[:])
            id96 = load("id96", [HID, HID], f16, id96_in[:])
            id128 = load("id128", [128, 128], f32, id128_in[:])
            idx_a = load("idx_a", [128, NWA * T_a * 8], dt.int16, idx_a_in[:])
            idx_b = load("idx_b", [128, NWA * T_b * 8], dt.int16, idx_b_in[:])
            dstloc = load("dstloc", [128, NWA, T], f16, dstloc_in[:])

            h_state = [pp.tile([HID, NPC], f16, name=f"h{s}")
                       for s in range(N_LAYERS + 1)]
            rm_buf = pp.tile([128, NW, 128], f16, name="rm_buf")
            # zero the pad columns once so gathered rows are NaN-free and
            # the aggregation matmul can use the full 128-col lhsT (FWL)
            nc.vector.memset(rm_buf[:, :, HID:128], 0.0)

            # double-buffered replicated tables: state s lives in buf s%2
            h_fullA = [dram.tile([CLS, 128], f16, name=f"h_fullA{i}",
                                 addr_space="Shared")
                       for i in range(N_LAYERS)]
            h_fullB = [dram.tile([N_NODES - CLS, 128], f16, name=f"h_fullB{i}",
                                 addr_space="Shared")
                       for i in range(N_LAYERS)]
            bounceA = dram.tile([HALF, 128], f16)
            bounceB = dram.tile([NPC - HALF, 128], f16)

            w_chunks = [(c0, min(CHUNK_W, NW - c0))
                        for c0 in range(0, NW, CHUNK_W)]

            def transpose_windows(s, w0, w1):
                for t in range(w0, w1):
                    n0 = t * 128
                    tn = min(128, NPC - n0)
                    pst = ps_tr.tile([128, HID], f16, name="pst")
                    nc.tensor.transpose(pst[:tn, :],
                                        h_state[s][:, n0:n0 + tn], id96[:])
                    nc.scalar.copy(rm_buf[:tn, t, 0:HID], pst[:tn, :])

            def bounce_chunk(j):
                # incremental bounce: ship chunk j's freshly transposed
                # windows to the DRAM staging buffer right away, so the
                # AllGather emitted later waits ~1us instead of a full
                # half-table DMA (AW is CHUNK_W-aligned: chunks 0..7 are
                # exactly the A half)
                c0, cw = w_chunks[j]
                if c0 < AW:
                    nc.sync.dma_start(
                        out=bounceA[c0 * WIN:(c0 + cw) * WIN, :].rearrange(
                            "(t p) d -> p t d", p=128),
                        in_=rm_buf[:, c0:c0 + cw, :])
                    return
                o0 = (c0 - AW) * WIN
                full = cw if c0 + cw < NW else cw - 1
                if full:
                    nc.sync.dma_start(
                        out=bounceB[o0:o0 + full * WIN, :].rearrange(
                            "(t p) d -> p t d", p=128),
                        in_=rm_buf[:, c0:c0 + full, :])
                if c0 + cw == NW:
                    last_n = NPC - (NW - 1) * WIN
                    o1 = o0 + full * WIN
                    nc.sync.dma_start(out=bounceB[o1:o1 + last_n, :],
                                      in_=rm_buf[0:last_n, NW - 1, :])

            def all_gather_a(s):
                nc.gpsimd.collective_compute(
                    "AllGather", mybir.AluOpType.bypass,
                    ins=[bounceA.opt()], outs=[h_fullA[s].opt()],
                    replica_groups=[list(range(N_CORES))])

            def all_gather_b(s):
                nc.gpsimd.collective_compute(
                    "AllGather", mybir.AluOpType.bypass,
                    ins=[bounceB.opt()], outs=[h_fullB[s].opt()],
                    replica_groups=[list(range(N_CORES))])

            qrr = [0]

            def emit_gathers(gbuf, src_view, idx_tile, base_tile, n_tiles):
                for s0 in range(0, n_tiles, GT):
                    sn = min(GT, n_tiles - s0)
                    nc.gpsimd.dma_gather(
                        gbuf[:, s0:s0 + sn, :], src_view,
                        idx_tile[:, (base_tile + s0) * 8:
                                 (base_tile + s0 + sn) * 8],
                        num_idxs=sn * 128, num_idxs_reg=sn * 128,
                        elem_size=128, single_packet=SINGLE_PACKET,
                        queue_num=qrr[0] % 4)
                    qrr[0] += 1

            # ---- input projection (chunk-pipelined epilogue) ----
            for j, (c0, cw) in enumerate(w_chunks):
                n0, cn = c0 * WIN, min(CW_N, NPC - c0 * WIN)
                xb = xpool.tile([IN_DIM, CW_N], f32r, name="xb")
                nc.sync.dma_start(out=xb[:, :cn],
                                  in_=xT_in[:, n0:n0 + cn].bitcast(f32r))
                ps = ps_big.tile([HID, CW_N], f32, name="psb")
                nc.tensor.matmul(ps[:, :cn], w_inT[:], xb[:, :cn],
                                 start=True, stop=True)
                nc.scalar.add(h_state[0][:, n0:n0 + cn], ps[:, :cn], b_in[:])
                transpose_windows(0, c0, c0 + cw)
                bounce_chunk(j)
                if j == BOUNCE_A_CHUNK:
                    all_gather_a(0)
            all_gather_b(0)

            # ---- GIN layers ----
            for l in range(N_LAYERS):
                tblA = h_fullA[l][:]
                tblB = h_fullB[l][:]
                ga_t, gb_t = {}, {}

                def emit_A(j, tblA=tblA, ga_t=ga_t):
                    c0a = j * APC
                    cwa = min(APC, NWA - c0a)
                    g = ga_pool.tile([128, APC * T_a, 128], f16,
                                     name="g_a")
                    emit_gathers(g, tblA, idx_a, c0a * T_a, cwa * T_a)
                    ga_t[j] = g

                def emit_B(j, tblB=tblB, gb_t=gb_t):
                    c0a = j * APC
                    cwa = min(APC, NWA - c0a)
                    g = gb_pool.tile([128, APC * T_b, 128], f16,
                                     name="g_b")
                    emit_gathers(g, tblB, idx_b, c0a * T_b, cwa * T_b)
                    gb_t[j] = g

                # The PE stream runs the A-class accumulation of chunk j+2
                # ahead of the B-class accumulation of chunk j: each chunk
                # owns a full PSUM bank whose per-window groups stay open
                # (A part stop=False) until the B tiles close them two
                # slots later.  At a layer boundary the PE therefore has
                # ~2 chunks of A work queued before the first instruction
                # that needs the (still in flight) AllGather-B table.
                ps_t = {}

                def a_chains(j, ps_t=ps_t, ga_t=ga_t):
                    c0a = j * APC
                    cwa = min(APC, NWA - c0a)
                    g_a = ga_t.pop(j)
                    ps = ps_agg.tile([128, CW_N], f32, name="psa")
                    ps_t[j] = ps
                    for wl in range(cwa):
                        w_i = c0a + wl
                        oha = oha_pool.tile([128, AGW, T_a], f16, name="oha")
                        nc.vector.tensor_tensor(
                            oha[:],
                            iota_wt[:, :, 0:T_a],
                            dstloc[:, w_i, 0:T_a].unsqueeze(1)
                                .broadcast_to([128, AGW, T_a]),
                            mybir.AluOpType.is_equal)
                        # full 128-col lhsT (pad cols are zeros) -> FWL
                        # fast weight load; only partitions 0:HID are read.
                        # start=True zeroes the WHOLE 2KB bank (zero-region
                        # granularity), so the bank is one accumulation
                        # group: start only on its very first matmul, stop
                        # only on its very last (in b_chains); unwritten
                        # elements first-write via has_written.
                        for t in range(T_a):
                            nc.tensor.matmul(
                                ps[:, wl * AGW:(wl + 1) * AGW],
                                g_a[:, wl * T_a + t, :], oha[:, :, t],
                                start=(wl == 0 and t == 0), stop=False,
                                skip_group_check=True)

                def b_chains(j, ps_t=ps_t, gb_t=gb_t):
                    c0a = j * APC
                    cwa = min(APC, NWA - c0a)
                    g_b = gb_t.pop(j)
                    ps = ps_t[j]
                    for wl in range(cwa):
                        w_i = c0a + wl
                        ohb = ohb_pool.tile([128, AGW, T_b], f16, name="ohb")
                        nc.vector.tensor_tensor(
                            ohb[:],
                            iota_wt[:, :, 0:T_b],
                            dstloc[:, w_i, T_a:T].unsqueeze(1)
                                .broadcast_to([128, AGW, T_b]),
                            mybir.AluOpType.is_equal)
                        for t in range(T_b):
                            nc.tensor.matmul(
                                ps[:, wl * AGW:(wl + 1) * AGW],
                                g_b[:, wl * T_b + t, :], ohb[:, :, t],
                                start=False,
                                stop=(wl == cwa - 1 and t == T_b - 1),
                                skip_group_check=True)

                emit_A(0)
                emit_A(1)
                emit_A(2)
                emit_B(0)
                emit_B(1)
                a_chains(0)
                a_chains(1)
                for j, (c0, cw) in enumerate(w_chunks):
                    if j + 3 < len(w_chunks):
                        emit_A(j + 3)
                    if j + 2 < len(w_chunks):
                        emit_B(j + 2)
                    b_chains(j)
                    # a_chains(j+2) keeps the PE busy while the DVE runs
                    # chunk j's +h add, so the dense matmul's input is
                    # ready the moment the PE reaches it
                    if j + 2 < len(w_chunks):
                        a_chains(j + 2)
                    n0, cn = c0 * WIN, min(CW_N, NPC - c0 * WIN)
                    ps = ps_t.pop(j)
                    hp = hp_pool.tile([HID, CW_N], f32r, name="hp")
                    nc.vector.tensor_tensor(
                        hp[:, :cn], ps[:HID, :cn],
                        h_state[l][:, n0:n0 + cn], mybir.AluOpType.add)
                    ps2 = ps_big.tile([HID, CW_N], f32, name="psb")
                    nc.tensor.matmul(ps2[:, :cn], w_lT[l][:], hp[:, :cn],
                                     start=True, stop=True)
                    nc.scalar.add(h_state[l + 1][:, n0:n0 + cn], ps2[:, :cn],
                                  b_l[l][:])
                    if l < N_LAYERS - 1:
                        transpose_windows(l + 1, c0, c0 + cw)
                        bounce_chunk(j)
                        if j == BOUNCE_A_CHUNK:
                            all_gather_a(l + 1)
                    else:
                        # interleave the output projection into layer 2
                        pso = ps_big.tile([OUT_DIM, CW_N], f32, name="pso",
                                          tag="psb")
                        for s in range(N_LAYERS + 1):
                            nc.tensor.matmul(pso[:, :cn], w_out4[s][:],
                                             h_state[s][:, n0:n0 + cn],
                                             start=(s == 0),
                                             stop=(s == N_LAYERS))
                        ot = ot_pool.tile([OUT_DIM, CW_N], f32, name="ot")
                        nc.scalar.add(ot[:, :cn], pso[:, :cn], b_out[:])
                        for tt in range(-(-cn // 128)):
                            t0 = tt * 128
                            tn = min(128, cn - t0)
                            pst = ps_tr.tile([128, 128], f32, name="psto",
                                             tag="pst")
                            nc.tensor.transpose(pst[:tn, :],
                                                ot[:, t0:t0 + tn], id128[:])
                            orow = ot_pool.tile([128, 128], f32, name="orow")
                            nc.scalar.copy(orow[:tn, :], pst[:tn, :])
                            nc.sync.dma_start(
                                out=out_ext[n0 + t0:n0 + t0 + tn, :],
                                in_=orow[:tn, :])
                if l < N_LAYERS - 1:
                    all_gather_b(l + 1)

    nc.compile()
    return nc


def _get_nc_and_inputs(inputs):
    from concourse import bass_utils  # noqa: F401  (path setup)

    x = np.asarray(inputs["x"], np.float32)
    edge_index = np.asarray(inputs["edge_index"], np.int32)
    W_in = np.asarray(inputs["W_in"], np.float32)
    b_in = np.asarray(inputs["b_in"], np.float32)
    W_layers = np.asarray(inputs["W_layers"], np.float32)
    b_layers = np.asarray(inputs["b_layers"], np.float32)
    W_out = np.asarray(inputs["W_out"], np.float32)
    b_out = np.asarray(inputs["b_out"], np.float32)

    idx_a_w, idx_b_w, dstloc, T_a, T_b, perm = _prep(edge_index)

    key = ("nc", T_a, T_b)
    if key not in _cache:
        _cache.clear()
        _cache[key] = _build(T_a, T_b)
    nc = _cache[key]

    T = T_a + T_b
    inv = np.empty(N_NODES, np.int64)
    inv[perm] = np.arange(N_NODES)
    xT = np.ascontiguousarray(x.T[:, inv])
    w_inT = np.ascontiguousarray(W_in.T)
    w_lT = np.ascontiguousarray(W_layers.transpose(0, 2, 1))
    b_l = np.ascontiguousarray(b_layers[:, :, None])
    w_out4 = np.ascontiguousarray(
        np.stack([W_out[:, s * HID:(s + 1) * HID].T
                  for s in range(N_LAYERS + 1)])).astype(np.float16)
    iota_wt = np.ascontiguousarray(np.broadcast_to(
        np.arange(AGW, dtype=np.float16)[None, :, None],
        (128, AGW, T)))
    id96 = np.eye(HID, dtype=np.float16)
    id128 = np.eye(128, dtype=np.float32)

    in_maps = []
    for c in range(N_CORES):
        in_maps.append({
            "xT": np.ascontiguousarray(xT[:, c * NPC:(c + 1) * NPC]),
            "w_inT": w_inT,
            "b_in": b_in.reshape(HID, 1),
            "w_lT": w_lT,
            "b_l": b_l,
            "w_out4": w_out4,
            "b_out": b_out.reshape(OUT_DIM, 1),
            "iota_wt": iota_wt,
            "id96": id96,
            "id128": id128,
            "idx_a": idx_a_w[c],
            "idx_b": idx_b_w[c],
            "dstloc": dstloc[c],
        })
    return nc, in_maps, perm


def run(inputs, trace=False):
    from concourse import bass_utils

    nc, in_maps, perm = _get_nc_and_inputs(inputs)
    res = bass_utils.run_bass_kernel_spmd(
        nc, in_maps, core_ids=list(range(N_CORES)), trace=trace)
    out = np.concatenate([res.results[c]["out"] for c in range(N_CORES)], 0)
    return out[perm], res


def kernel(**inputs):
    out, _ = run(inputs, trace=False)
    return out



# revision 30
# speedup vs baseline: 1.4160x; 1.4160x over previous
"""GIN-style 3-layer GNN encoder on 8 Trainium2 NeuronCores (Bass/Tile).

Reference computation (fp32):
    h = x @ W_in.T + b_in                                  [50000, 96]
    for l in 0..2:
        agg = segment_sum(h[src], dst, N)                  [50000, 96]
        h = (h + agg) @ W_layers[l].T + b_layers[l]
    out = concat([h0..h3], 1) @ W_out.T + b_out            [50000, 128]

Distribution: nodes are partitioned across the 8 cores (6250/core) via a
host-side balancing permutation; each edge is owned by the core that owns
its dst node.  Each layer the updated node features are AllGathered into
two replicated row-major fp16 tables h_fullA/h_fullB (first/second half
of every core's node range, 25000 x 256B rows each, one Shared-space
pair per state so collectives write once and never alias) — the split
halves the AllGather latency on the critical path and keeps gather
indices < 32768 (int16).

Per-core segment sum: a core's node range is split into 49 windows of 128
nodes.  Every window has a fixed number of 128-edge tiles (T_a tiles with
src in half A, T_b in half B; the balancing permutation equalizes
per-window per-class edge counts so the fixed tile counts are tight).
Edge features are fetched with gpsimd dma_gather (fp16 256B rows, 1024
idxs per instruction, round-robin over the 4 SWDGE queues).  For each
window the one-hot onehot[e, j, t] = (j == dst_local[e, t]) is built on
DVE (layout [128, WIN, T] keeps every operand's last dim stride-1 so the
2x DVE perf mode engages), and the PE accumulates
    psum[96, 128] += gathered_tile[128e, 96].T @ onehot[:, :, t]
which is aggT for the window.

The whole layer is chunk-pipelined: each chunk of 4 windows (512 nodes =
one PSUM bank) flows gathers -> onehot/agg -> +h (DVE) -> layer matmul ->
bias -> PE transpose to the row-major fp16 shard.  bounceA DMAs fire
mid-layer (after window 24) and AllGather-A for the next table is emitted
mid-gather-stream (chunk 9) so its flight overlaps the rest of the layer;
AllGather-B fires at layer end (a collective's sequencer wait only blocks
until its input is ready — the flight itself runs async on the CC cores).
The final output projection is interleaved into layer 2's chunk loop.
"""
import sys

sys.path.insert(0, "/opt/trn_rl_repo")

import numpy as np

N_NODES = 50000
N_EDGES = 800000
IN_DIM = 128
HID = 96
OUT_DIM = 128
N_LAYERS = 3
N_CORES = 8
NPC = N_NODES // N_CORES          # 6250 nodes per core
WIN = 128                         # window width (nodes)
NW = (NPC + WIN - 1) // WIN       # 49 windows per core (last = 106 nodes)
HALF = 4096                       # per-core A/B split: A = first 4096 nodes
                                  # (global A = 32768 rows = int16 limit);
                                  # asymmetric so AllGather-B is small and
                                  # lands quickly at the layer boundary
BHALF = NPC - HALF                # 2154 B nodes per core
CLS = N_CORES * HALF              # 32768: A-class size
AW = HALF // WIN                  # 32 full-A windows per core
REM_A = HALF - AW * WIN           # 0 A-slots in window AW
CHUNK_W = 4                       # windows per chunk (= 512 nodes = 1 bank)
AGW = 128                         # aggregation window width (PE onehot
                                  # matmul free dim)
NWA = (NPC + AGW - 1) // AGW      # 98 agg windows per core (last = 42)
APC = CHUNK_W * WIN // AGW        # 8 agg windows per chunk
GT = 8                            # tiles per dma_gather (1024 idxs = the
                                  # 64-descs-per-engine single_packet cap)
SINGLE_PACKET = True
CW_N = CHUNK_W * WIN              # 512: node-chunk for dense matmuls
BOUNCE_A_CHUNK = (AW * WIN + REM_A - 1) // CW_N   # chunk whose transposes
                                                  # complete the A half (6)

_cache = {}


def _balance_nodes(src0, dst0):
    """Permute node ids so per-(core,window) A/B edge counts are even.

    A node's A/B class (which replicated gather table its row lives in) is
    frozen to its OLD id (< CLS -> A); the permutation only moves nodes
    within their class region, so per-node (deg_a, deg_b) are fixed and a
    greedy 2-D balance over the 392 (core, window) bins makes the uniform
    tile counts T_a/T_b tight.  Returns perm (old id -> new id).
    """
    deg_a = np.bincount(dst0[src0 < CLS], minlength=N_NODES).astype(np.int64)
    deg_b = np.bincount(dst0[src0 >= CLS], minlength=N_NODES).astype(np.int64)
    nbins = N_CORES * NWA
    base = np.empty(nbins, np.int64)
    cap = np.empty(nbins, np.int64)
    for b in range(nbins):
        c, w = divmod(b, NWA)
        base[b] = c * NPC + w * AGW
        cap[b] = min(AGW, NPC - w * AGW)
    woff = base % NPC
    q_a = np.clip(HALF - woff, 0, cap)   # A slots = first q_a of the window
    q_b = cap - q_a

    mu_a = max(1.0, deg_a.sum() / nbins)
    mu_b = max(1.0, deg_b.sum() / nbins)
    order = np.argsort(-(deg_a + deg_b), kind="stable")
    a_load = np.zeros(nbins)
    b_load = np.zeros(nbins)
    a_left = q_a.copy()
    b_left = q_b.copy()
    a_pos = np.zeros(nbins, np.int64)
    b_pos = q_a.copy()
    perm = np.empty(N_NODES, np.int64)
    for n in order:
        phi = np.maximum((a_load + deg_a[n]) / mu_a,
                         (b_load + deg_b[n]) / mu_b)
        if n < CLS:
            phi = np.where(a_left > 0, phi, np.inf)
            b_ = int(np.argmin(phi))
            perm[n] = base[b_] + a_pos[b_]
            a_pos[b_] += 1
            a_left[b_] -= 1
        else:
            phi = np.where(b_left > 0, phi, np.inf)
            b_ = int(np.argmin(phi))
            perm[n] = base[b_] + b_pos[b_]
            b_pos[b_] += 1
            b_left[b_] -= 1
        a_load[b_] += deg_a[n]
        b_load[b_] += deg_b[n]
    return perm


def _prep(edge_index):
    """Host-side edge bucketing -> per-core gather index / dst tables."""
    src0 = edge_index[0].astype(np.int64)
    dst0 = edge_index[1].astype(np.int64)
    perm = _balance_nodes(src0, dst0)
    src = perm[src0]
    dst = perm[dst0]
    core = dst // NPC
    din = dst % NPC
    w = din // AGW
    dstl = din % AGW
    s_in = src % NPC
    c_src = src // NPC
    is_b = (s_in >= HALF).astype(np.int64)
    pos = np.where(is_b == 0, c_src * HALF + s_in,
                   c_src * BHALF + s_in - HALF)  # < 32768, int16-safe

    key = (core * NWA + w) * 2 + is_b
    order = np.argsort(key, kind="stable")
    s_pos = pos[order]
    s_dstl = dstl[order]
    s_key = key[order]
    s_b = is_b[order]

    counts = np.bincount(key, minlength=N_CORES * NWA * 2)
    T_a = max(1, int(-(-counts.reshape(-1, 2)[:, 0].max() // 128)))
    T_b = max(1, int(-(-counts.reshape(-1, 2)[:, 1].max() // 128)))
    T = T_a + T_b

    starts = np.zeros(N_CORES * NWA * 2, np.int64)
    starts[1:] = np.cumsum(counts)[:-1]
    rank = np.arange(len(s_key)) - starts[s_key]

    c_arr = s_key // (2 * NWA)
    w_arr = (s_key // 2) % NWA

    idx_a = np.zeros((N_CORES, NWA, T_a * 128), np.int16)
    idx_b = np.zeros((N_CORES, NWA, T_b * 128), np.int16)
    dstl_arr = np.full((N_CORES, NWA, T, 128), -1.0, np.float16)

    a_m = s_b == 0
    flat = (c_arr[a_m] * NWA + w_arr[a_m]) * (T_a * 128) + rank[a_m]
    idx_a.reshape(-1)[flat] = s_pos[a_m].astype(np.int16)
    t_g = rank[a_m] // 128
    e_g = rank[a_m] % 128
    flat = ((c_arr[a_m] * NWA + w_arr[a_m]) * T + t_g) * 128 + e_g
    dstl_arr.reshape(-1)[flat] = s_dstl[a_m].astype(np.float16)

    b_m = ~a_m
    flat = (c_arr[b_m] * NWA + w_arr[b_m]) * (T_b * 128) + rank[b_m]
    idx_b.reshape(-1)[flat] = s_pos[b_m].astype(np.int16)
    t_g = rank[b_m] // 128 + T_a
    e_g = rank[b_m] % 128
    flat = ((c_arr[b_m] * NWA + w_arr[b_m]) * T + t_g) * 128 + e_g
    dstl_arr.reshape(-1)[flat] = s_dstl[b_m].astype(np.float16)

    def wrap(vals):  # [NW*Tc*128] -> [128, NW*Tc*8] int16 wrapped+replicated
        v = vals.reshape(-1, 16).T
        return np.tile(v, (8, 1)).copy()

    idx_a_w = np.stack([wrap(idx_a[c].reshape(-1)) for c in range(N_CORES)])
    idx_b_w = np.stack([wrap(idx_b[c].reshape(-1)) for c in range(N_CORES)])
    dstloc = np.ascontiguousarray(dstl_arr.transpose(0, 3, 1, 2))  # [C,128,NW,T]
    return idx_a_w, idx_b_w, dstloc, T_a, T_b, perm


def _build(T_a, T_b):
    from concourse import bacc, tile, mybir, library_config

    dt = mybir.dt
    T = T_a + T_b
    nc = bacc.Bacc("TRN2", target_bir_lowering=False, debug=False,
                   num_devices=N_CORES, num_swdge_queues=4,
                   dynamic_dma_scratch_size=49152)

    # ---- I/O ----
    xT_in = nc.dram_tensor("xT", [IN_DIM, NPC], dt.float32, kind="ExternalInput")
    w_inT_in = nc.dram_tensor("w_inT", [IN_DIM, HID], dt.float32,
                              kind="ExternalInput")
    b_in_in = nc.dram_tensor("b_in", [HID, 1], dt.float32, kind="ExternalInput")
    w_lT_in = nc.dram_tensor("w_lT", [N_LAYERS, HID, HID], dt.float32,
                             kind="ExternalInput")
    b_l_in = nc.dram_tensor("b_l", [N_LAYERS, HID, 1], dt.float32,
                            kind="ExternalInput")
    w_out4_in = nc.dram_tensor("w_out4", [N_LAYERS + 1, HID, OUT_DIM],
                               dt.float16, kind="ExternalInput")
    b_out_in = nc.dram_tensor("b_out", [OUT_DIM, 1], dt.float32,
                              kind="ExternalInput")
    iota_in = nc.dram_tensor("iota_wt", [128, AGW, T], dt.float16,
                             kind="ExternalInput")
    id96_in = nc.dram_tensor("id96", [HID, HID], dt.float16,
                             kind="ExternalInput")
    id128_in = nc.dram_tensor("id128", [128, 128], dt.float32,
                              kind="ExternalInput")
    idx_a_in = nc.dram_tensor("idx_a", [128, NWA * T_a * 8], dt.int16,
                              kind="ExternalInput")
    idx_b_in = nc.dram_tensor("idx_b", [128, NWA * T_b * 8], dt.int16,
                              kind="ExternalInput")
    dstloc_in = nc.dram_tensor("dstloc", [128, NWA, T], dt.float16,
                               kind="ExternalInput")
    out_ext = nc.dram_tensor("out", [NPC, OUT_DIM], dt.float32,
                             kind="ExternalOutput")

    f32, f32r, f16 = dt.float32, dt.float32r, dt.float16

    with tile.TileContext(nc, num_cores=N_CORES) as tc:
        nc.gpsimd.load_library(library_config.mlp)
        with tc.tile_pool(name="persist", bufs=1) as pp, \
             tc.tile_pool(name="xpool", bufs=2) as xpool, \
             tc.tile_pool(name="hp", bufs=3) as hp_pool, \
             tc.tile_pool(name="ga", bufs=3) as ga_pool, \
             tc.tile_pool(name="gb", bufs=3) as gb_pool, \
             tc.tile_pool(name="oha", bufs=6) as oha_pool, \
             tc.tile_pool(name="ohb", bufs=6) as ohb_pool, \
             tc.tile_pool(name="otile", bufs=2) as ot_pool, \
             tc.tile_pool(name="ps_agg", bufs=3, space="PSUM") as ps_agg, \
             tc.tile_pool(name="ps_big", bufs=2, space="PSUM") as ps_big, \
             tc.tile_pool(name="ps_tr", bufs=2, space="PSUM") as ps_tr, \
             tc.tile_pool(name="dram", bufs=1, space="DRAM") as dram:

            def load(name, shape, dtype, src_ap):
                t = pp.tile(shape, dtype, name=name)
                nc.sync.dma_start(out=t[:], in_=src_ap)
                return t

            w_inT = load("w_inT", [IN_DIM, HID], f32r, w_inT_in[:].bitcast(f32r))
            b_in = load("b_in", [HID, 1], f32, b_in_in[:])
            w_lT = [load(f"w_lT{l}", [HID, HID], f32r, w_lT_in[l].bitcast(f32r))
                    for l in range(N_LAYERS)]
            b_l = [load(f"b_l{l}", [HID, 1], f32, b_l_in[l])
                   for l in range(N_LAYERS)]
            w_out4 = [load(f"w_out4_{s}", [HID, OUT_DIM], f16, w_out4_in[s])
                      for s in range(N_LAYERS + 1)]
            b_out = load("b_out", [OUT_DIM, 1], f32, b_out_in[:])
            iota_wt = load("iota_wt", [128, AGW, T], f16, iota_in[:])
            id96 = load("id96", [HID, HID], f16, id96_in[:])
            id128 = load("id128", [128, 128], f32, id128_in[:])
            idx_a = load("idx_a", [128, NWA * T_a * 8], dt.int16, idx_a_in[:])
            idx_b = load("idx_b", [128, NWA * T_b * 8], dt.int16, idx_b_in[:])
            dstloc = load("dstloc", [128, NWA, T], f16, dstloc_in[:])

            h_state = [pp.tile([HID, NPC], f16, name=f"h{s}")
                       for s in range(N_LAYERS + 1)]
            rm_buf = pp.tile([128, NW, 128], f16, name="rm_buf")
            # zero the pad columns once so gathered rows are NaN-free and
            # the aggregation matmul can use the full 128-col lhsT (FWL)
            nc.vector.memset(rm_buf[:, :, HID:128], 0.0)

            # double-buffered replicated tables: state s lives in buf s%2
            h_fullA = [dram.tile([CLS, 128], f16, name=f"h_fullA{i}",
                                 addr_space="Shared")
                       for i in range(N_LAYERS)]
            h_fullB = [dram.tile([N_NODES - CLS, 128], f16, name=f"h_fullB{i}",
                                 addr_space="Shared")
                       for i in range(N_LAYERS)]
            bounceA = dram.tile([HALF, 128], f16)
            bounceB = dram.tile([NPC - HALF, 128], f16)

            w_chunks = [(c0, min(CHUNK_W, NW - c0))
                        for c0 in range(0, NW, CHUNK_W)]

            def transpose_windows(s, w0, w1):
                for t in range(w0, w1):
                    n0 = t * 128
                    tn = min(128, NPC - n0)
                    pst = ps_tr.tile([128, HID], f16, name="pst")
                    nc.tensor.transpose(pst[:tn, :],
                                        h_state[s][:, n0:n0 + tn], id96[:])
                    nc.scalar.copy(rm_buf[:tn, t, 0:HID], pst[:tn, :])

            def bounce_chunk(j):
                # incremental bounce: ship chunk j's freshly transposed
                # windows to the DRAM staging buffer right away, so the
                # AllGather emitted later waits ~1us instead of a full
                # half-table DMA (AW is CHUNK_W-aligned: chunks 0..7 are
                # exactly the A half)
                c0, cw = w_chunks[j]
                if c0 < AW:
                    nc.sync.dma_start(
                        out=bounceA[c0 * WIN:(c0 + cw) * WIN, :].rearrange(
                            "(t p) d -> p t d", p=128),
                        in_=rm_buf[:, c0:c0 + cw, :])
                    return
                o0 = (c0 - AW) * WIN
                full = cw if c0 + cw < NW else cw - 1
                if full:
                    nc.sync.dma_start(
                        out=bounceB[o0:o0 + full * WIN, :].rearrange(
                            "(t p) d -> p t d", p=128),
                        in_=rm_buf[:, c0:c0 + full, :])
                if c0 + cw == NW:
                    last_n = NPC - (NW - 1) * WIN
                    o1 = o0 + full * WIN
                    nc.sync.dma_start(out=bounceB[o1:o1 + last_n, :],
                                      in_=rm_buf[0:last_n, NW - 1, :])

            def all_gather_a(s):
                nc.gpsimd.collective_compute(
                    "AllGather", mybir.AluOpType.bypass,
                    ins=[bounceA.opt()], outs=[h_fullA[s].opt()],
                    replica_groups=[list(range(N_CORES))])

            def all_gather_b(s):
                nc.gpsimd.collective_compute(
                    "AllGather", mybir.AluOpType.bypass,
                    ins=[bounceB.opt()], outs=[h_fullB[s].opt()],
                    replica_groups=[list(range(N_CORES))])

            qrr = [0]

            def emit_gathers(gbuf, src_view, idx_tile, base_tile, n_tiles):
                for s0 in range(0, n_tiles, GT):
                    sn = min(GT, n_tiles - s0)
                    nc.gpsimd.dma_gather(
                        gbuf[:, s0:s0 + sn, :], src_view,
                        idx_tile[:, (base_tile + s0) * 8:
                                 (base_tile + s0 + sn) * 8],
                        num_idxs=sn * 128, num_idxs_reg=sn * 128,
                        elem_size=128, single_packet=SINGLE_PACKET,
                        queue_num=qrr[0] % 4)
                    qrr[0] += 1

            # ---- input projection (chunk-pipelined epilogue) ----
            for j, (c0, cw) in enumerate(w_chunks):
                n0, cn = c0 * WIN, min(CW_N, NPC - c0 * WIN)
                xb = xpool.tile([IN_DIM, CW_N], f32r, name="xb")
                nc.sync.dma_start(out=xb[:, :cn],
                                  in_=xT_in[:, n0:n0 + cn].bitcast(f32r))
                ps = ps_big.tile([HID, CW_N], f32, name="psb")
                nc.tensor.matmul(ps[:, :cn], w_inT[:], xb[:, :cn],
                                 start=True, stop=True)
                nc.scalar.add(h_state[0][:, n0:n0 + cn], ps[:, :cn], b_in[:])
                transpose_windows(0, c0, c0 + cw)
                bounce_chunk(j)
                if j == BOUNCE_A_CHUNK:
                    all_gather_a(0)
            all_gather_b(0)

            # ---- GIN layers ----
            for l in range(N_LAYERS):
                tblA = h_fullA[l][:]
                tblB = h_fullB[l][:]
                ga_t, gb_t = {}, {}

                def emit_A(j, tblA=tblA, ga_t=ga_t):
                    c0a = j * APC
                    cwa = min(APC, NWA - c0a)
                    g = ga_pool.tile([128, APC * T_a, 128], f16,
                                     name="g_a")
                    emit_gathers(g, tblA, idx_a, c0a * T_a, cwa * T_a)
                    ga_t[j] = g

                def emit_B(j, tblB=tblB, gb_t=gb_t):
                    c0a = j * APC
                    cwa = min(APC, NWA - c0a)
                    g = gb_pool.tile([128, APC * T_b, 128], f16,
                                     name="g_b")
                    emit_gathers(g, tblB, idx_b, c0a * T_b, cwa * T_b)
                    gb_t[j] = g

                # The PE stream runs the A-class accumulation of chunk j+2
                # ahead of the B-class accumulation of chunk j: each chunk
                # owns a full PSUM bank whose per-window groups stay open
                # (A part stop=False) until the B tiles close them two
                # slots later.  At a layer boundary the PE therefore has
                # ~2 chunks of A work queued before the first instruction
                # that needs the (still in flight) AllGather-B table.
                ps_t = {}

                def a_chains(j, ps_t=ps_t, ga_t=ga_t):
                    c0a = j * APC
                    cwa = min(APC, NWA - c0a)
                    g_a = ga_t.pop(j)
                    ps = ps_agg.tile([128, CW_N], f32, name="psa")
                    ps_t[j] = ps
                    for wl in range(cwa):
                        w_i = c0a + wl
                        oha = oha_pool.tile([128, AGW, T_a], f16, name="oha")
                        nc.vector.tensor_tensor(
                            oha[:],
                            iota_wt[:, :, 0:T_a],
                            dstloc[:, w_i, 0:T_a].unsqueeze(1)
                                .broadcast_to([128, AGW, T_a]),
                            mybir.AluOpType.is_equal)
                        # full 128-col lhsT (pad cols are zeros) -> FWL
                        # fast weight load; only partitions 0:HID are read.
                        # start=True zeroes the WHOLE 2KB bank (zero-region
                        # granularity), so the bank is one accumulation
                        # group: start only on its very first matmul, stop
                        # only on its very last (in b_chains); unwritten
                        # elements first-write via has_written.
                        for t in range(T_a):
                            nc.tensor.matmul(
                                ps[:, wl * AGW:(wl + 1) * AGW],
                                g_a[:, wl * T_a + t, :], oha[:, :, t],
                                start=(wl == 0 and t == 0), stop=False,
                                skip_group_check=True)

                def b_chains(j, ps_t=ps_t, gb_t=gb_t):
                    c0a = j * APC
                    cwa = min(APC, NWA - c0a)
                    g_b = gb_t.pop(j)
                    ps = ps_t[j]
                    for wl in range(cwa):
                        w_i = c0a + wl
                        ohb = ohb_pool.tile([128, AGW, T_b], f16, name="ohb")
                        nc.vector.tensor_tensor(
                            ohb[:],
                            iota_wt[:, :, 0:T_b],
                            dstloc[:, w_i, T_a:T].unsqueeze(1)
                                .broadcast_to([128, AGW, T_b]),
                            mybir.AluOpType.is_equal)
                        for t in range(T_b):
                            nc.tensor.matmul(
                                ps[:, wl * AGW:(wl + 1) * AGW],
                                g_b[:, wl * T_b + t, :], ohb[:, :, t],
                                start=False,
                                stop=(wl == cwa - 1 and t == T_b - 1),
                                skip_group_check=True)

                emit_A(0)
                emit_A(1)
                emit_A(2)
                emit_B(0)
                emit_B(1)
                a_chains(0)
                a_chains(1)
                for j, (c0, cw) in enumerate(w_chunks):
                    if j + 3 < len(w_chunks):
                        emit_A(j + 3)
                    if j + 2 < len(w_chunks):
                        emit_B(j + 2)
                    b_chains(j)
                    # a_chains(j+2) keeps the PE busy while the DVE runs
                    # chunk j's +h add, so the dense matmul's input is
                    # ready the moment the PE reaches it
                    if j + 2 < len(w_chunks):
                        a_chains(j + 2)
                    n0, cn = c0 * WIN, min(CW_N, NPC - c0 * WIN)
                    ps = ps_t.pop(j)
                    hp = hp_pool.tile([HID, CW_N], f32r, name="hp")
                    nc.vector.tensor_tensor(
                        hp[:, :cn], ps[:HID, :cn],
                        h_state[l][:, n0:n0 + cn], mybir.AluOpType.add)
                    ps2 = ps_big.tile([HID, CW_N], f32, name="psb")
                    nc.tensor.matmul(ps2[:, :cn], w_lT[l][:], hp[:, :cn],
                                     start=True, stop=True)
                    nc.scalar.add(h_state[l + 1][:, n0:n0 + cn], ps2[:, :cn],
                                  b_l[l][:])
                    if l < N_LAYERS - 1:
                        transpose_windows(l + 1, c0, c0 + cw)
                        bounce_chunk(j)
                        if j == BOUNCE_A_CHUNK:
                            all_gather_a(l + 1)
                    else:
                        # interleave the output projection into layer 2
                        pso = ps_big.tile([OUT_DIM, CW_N], f32, name="pso",
                                          tag="psb")
                        for s in range(N_LAYERS + 1):
                            nc.tensor.matmul(pso[:, :cn], w_out4[s][:],
                                             h_state[s][:, n0:n0 + cn],
                                             start=(s == 0),
                                             stop=(s == N_LAYERS))
                        ot = ot_pool.tile([OUT_DIM, CW_N], f32, name="ot")
                        nc.scalar.add(ot[:, :cn], pso[:, :cn], b_out[:])
                        for tt in range(-(-cn // 128)):
                            t0 = tt * 128
                            tn = min(128, cn - t0)
                            pst = ps_tr.tile([128, 128], f32, name="psto",
                                             tag="pst")
                            nc.tensor.transpose(pst[:tn, :],
                                                ot[:, t0:t0 + tn], id128[:])
                            orow = ot_pool.tile([128, 128], f32, name="orow")
                            nc.scalar.copy(orow[:tn, :], pst[:tn, :])
                            nc.sync.dma_start(
                                out=out_ext[n0 + t0:n0 + t0 + tn, :],
                                in_=orow[:tn, :])
                if l < N_LAYERS - 1:
                    all_gather_b(l + 1)

    nc.compile()
    return nc


def _get_nc_and_inputs(inputs):
    from concourse import bass_utils  # noqa: F401  (path setup)

    x = np.asarray(inputs["x"], np.float32)
    edge_index = np.asarray(inputs["edge_index"], np.int32)
    W_in = np.asarray(inputs["W_in"], np.float32)
    b_in = np.asarray(inputs["b_in"], np.float32)
    W_layers = np.asarray(inputs["W_layers"], np.float32)
    b_layers = np.asarray(inputs["b_layers"], np.float32)
    W_out = np.asarray(inputs["W_out"], np.float32)
    b_out = np.asarray(inputs["b_out"], np.float32)

    idx_a_w, idx_b_w, dstloc, T_a, T_b, perm = _prep(edge_index)

    key = ("nc", T_a, T_b)
    if key not in _cache:
        _cache.clear()
        _cache[key] = _build(T_a, T_b)
    nc = _cache[key]

    T = T_a + T_b
    inv = np.empty(N_NODES, np.int64)
    inv[perm] = np.arange(N_NODES)
    xT = np.ascontiguousarray(x.T[:, inv])
    w_inT = np.ascontiguousarray(W_in.T)
    w_lT = np.ascontiguousarray(W_layers.transpose(0, 2, 1))
    b_l = np.ascontiguousarray(b_layers[:, :, None])
    w_out4 = np.ascontiguousarray(
        np.stack([W_out[:, s * HID:(s + 1) * HID].T
                  for s in range(N_LAYERS + 1)])).astype(np.float16)
    iota_wt = np.ascontiguousarray(np.broadcast_to(
        np.arange(AGW, dtype=np.float16)[None, :, None],
        (128, AGW, T)))
    id96 = np.eye(HID, dtype=np.float16)
    id128 = np.eye(128, dtype=np.float32)

    in_maps = []
    for c in range(N_CORES):
        in_maps.append({
            "xT": np.ascontiguousarray(xT[:, c * NPC:(c + 1) * NPC]),
            "w_inT": w_inT,
            "b_in": b_in.reshape(HID, 1),
            "w_lT": w_lT,
            "b_l": b_l,
            "w_out4": w_out4,
            "b_out": b_out.reshape(OUT_DIM, 1),
            "iota_wt": iota_wt,
            "id96": id96,
            "id128": id128,
            "idx_a": idx_a_w[c],
            "idx_b": idx_b_w[c],
            "dstloc": dstloc[c],
        })
    return nc, in_maps, perm


def run(inputs, trace=False):
    from concourse import bass_utils

    nc, in_maps, perm = _get_nc_and_inputs(inputs)
    res = bass_utils.run_bass_kernel_spmd(
        nc, in_maps, core_ids=list(range(N_CORES)), trace=trace)
    out = np.concatenate([res.results[c]["out"] for c in range(N_CORES)], 0)
    return out[perm], res


def kernel(**inputs):
    out, _ = run(inputs, trace=False)
    return out



# revision 32
# speedup vs baseline: 1.4234x; 1.0052x over previous
"""GIN-style 3-layer GNN encoder on 8 Trainium2 NeuronCores (Bass/Tile).

Reference computation (fp32):
    h = x @ W_in.T + b_in                                  [50000, 96]
    for l in 0..2:
        agg = segment_sum(h[src], dst, N)                  [50000, 96]
        h = (h + agg) @ W_layers[l].T + b_layers[l]
    out = concat([h0..h3], 1) @ W_out.T + b_out            [50000, 128]

Distribution: nodes are partitioned across the 8 cores (6250/core) via a
host-side balancing permutation; each edge is owned by the core that owns
its dst node.  Each layer the updated node features are AllGathered into
two replicated row-major fp16 tables h_fullA/h_fullB (first/second half
of every core's node range, 25000 x 256B rows each, one Shared-space
pair per state so collectives write once and never alias) — the split
halves the AllGather latency on the critical path and keeps gather
indices < 32768 (int16).

Per-core segment sum: a core's node range is split into 49 windows of 128
nodes.  Every window has a fixed number of 128-edge tiles (T_a tiles with
src in half A, T_b in half B; the balancing permutation equalizes
per-window per-class edge counts so the fixed tile counts are tight).
Edge features are fetched with gpsimd dma_gather (fp16 256B rows, 1024
idxs per instruction, round-robin over the 4 SWDGE queues).  For each
window the one-hot onehot[e, j, t] = (j == dst_local[e, t]) is built on
DVE (layout [128, WIN, T] keeps every operand's last dim stride-1 so the
2x DVE perf mode engages), and the PE accumulates
    psum[96, 128] += gathered_tile[128e, 96].T @ onehot[:, :, t]
which is aggT for the window.

The whole layer is chunk-pipelined: each chunk of 4 windows (512 nodes =
one PSUM bank) flows gathers -> onehot/agg -> +h (DVE) -> layer matmul ->
bias -> PE transpose to the row-major fp16 shard.  bounceA DMAs fire
mid-layer (after window 24) and AllGather-A for the next table is emitted
mid-gather-stream (chunk 9) so its flight overlaps the rest of the layer;
AllGather-B fires at layer end (a collective's sequencer wait only blocks
until its input is ready — the flight itself runs async on the CC cores).
The final output projection is interleaved into layer 2's chunk loop.
"""
import sys

sys.path.insert(0, "/opt/trn_rl_repo")

import numpy as np

N_NODES = 50000
N_EDGES = 800000
IN_DIM = 128
HID = 96
OUT_DIM = 128
N_LAYERS = 3
N_CORES = 8
NPC = N_NODES // N_CORES          # 6250 nodes per core
WIN = 128                         # window width (nodes)
NW = (NPC + WIN - 1) // WIN       # 49 windows per core (last = 106 nodes)
HALF = 4096                       # per-core A/B split: A = first 4096 nodes
                                  # (global A = 32768 rows = int16 limit);
                                  # asymmetric so AllGather-B is small and
                                  # lands quickly at the layer boundary
BHALF = NPC - HALF                # 2154 B nodes per core
CLS = N_CORES * HALF              # 32768: A-class size
AW = HALF // WIN                  # 32 full-A windows per core
REM_A = HALF - AW * WIN           # 0 A-slots in window AW
CHUNK_W = 4                       # windows per chunk (= 512 nodes = 1 bank)
AGW = 128                         # aggregation window width (PE onehot
                                  # matmul free dim)
NWA = (NPC + AGW - 1) // AGW      # 98 agg windows per core (last = 42)
APC = CHUNK_W * WIN // AGW        # 8 agg windows per chunk
GT = 8                            # tiles per dma_gather (1024 idxs = the
                                  # 64-descs-per-engine single_packet cap)
SINGLE_PACKET = True
CW_N = CHUNK_W * WIN              # 512: node-chunk for dense matmuls
BOUNCE_A_CHUNK = (AW * WIN + REM_A - 1) // CW_N   # chunk whose transposes
                                                  # complete the A half (6)

_cache = {}


def _balance_nodes(src0, dst0):
    """Permute node ids so per-(core,window) A/B edge counts are even.

    A node's A/B class (which replicated gather table its row lives in) is
    frozen to its OLD id (< CLS -> A); the permutation only moves nodes
    within their class region, so per-node (deg_a, deg_b) are fixed and a
    greedy 2-D balance over the 392 (core, window) bins makes the uniform
    tile counts T_a/T_b tight.  Returns perm (old id -> new id).
    """
    deg_a = np.bincount(dst0[src0 < CLS], minlength=N_NODES).astype(np.int64)
    deg_b = np.bincount(dst0[src0 >= CLS], minlength=N_NODES).astype(np.int64)
    nbins = N_CORES * NWA
    base = np.empty(nbins, np.int64)
    cap = np.empty(nbins, np.int64)
    for b in range(nbins):
        c, w = divmod(b, NWA)
        base[b] = c * NPC + w * AGW
        cap[b] = min(AGW, NPC - w * AGW)
    woff = base % NPC
    q_a = np.clip(HALF - woff, 0, cap)   # A slots = first q_a of the window
    q_b = cap - q_a

    mu_a = max(1.0, deg_a.sum() / nbins)
    mu_b = max(1.0, deg_b.sum() / nbins)
    order = np.argsort(-(deg_a + deg_b), kind="stable")
    a_load = np.zeros(nbins)
    b_load = np.zeros(nbins)
    a_left = q_a.copy()
    b_left = q_b.copy()
    a_pos = np.zeros(nbins, np.int64)
    b_pos = q_a.copy()
    perm = np.empty(N_NODES, np.int64)
    for n in order:
        phi = np.maximum((a_load + deg_a[n]) / mu_a,
                         (b_load + deg_b[n]) / mu_b)
        if n < CLS:
            phi = np.where(a_left > 0, phi, np.inf)
            b_ = int(np.argmin(phi))
            perm[n] = base[b_] + a_pos[b_]
            a_pos[b_] += 1
            a_left[b_] -= 1
        else:
            phi = np.where(b_left > 0, phi, np.inf)
            b_ = int(np.argmin(phi))
            perm[n] = base[b_] + b_pos[b_]
            b_pos[b_] += 1
            b_left[b_] -= 1
        a_load[b_] += deg_a[n]
        b_load[b_] += deg_b[n]
    return perm


def _prep(edge_index):
    """Host-side edge bucketing -> per-core gather index / dst tables."""
    src0 = edge_index[0].astype(np.int64)
    dst0 = edge_index[1].astype(np.int64)
    perm = _balance_nodes(src0, dst0)
    src = perm[src0]
    dst = perm[dst0]
    core = dst // NPC
    din = dst % NPC
    w = din // AGW
    dstl = din % AGW
    s_in = src % NPC
    c_src = src // NPC
    is_b = (s_in >= HALF).astype(np.int64)
    pos = np.where(is_b == 0, c_src * HALF + s_in,
                   c_src * BHALF + s_in - HALF)  # < 32768, int16-safe

    key = (core * NWA + w) * 2 + is_b
    order = np.argsort(key, kind="stable")
    s_pos = pos[order]
    s_dstl = dstl[order]
    s_key = key[order]
    s_b = is_b[order]

    counts = np.bincount(key, minlength=N_CORES * NWA * 2)
    T_a = max(1, int(-(-counts.reshape(-1, 2)[:, 0].max() // 128)))
    T_b = max(1, int(-(-counts.reshape(-1, 2)[:, 1].max() // 128)))
    T = T_a + T_b

    starts = np.zeros(N_CORES * NWA * 2, np.int64)
    starts[1:] = np.cumsum(counts)[:-1]
    rank = np.arange(len(s_key)) - starts[s_key]

    c_arr = s_key // (2 * NWA)
    w_arr = (s_key // 2) % NWA

    idx_a = np.zeros((N_CORES, NWA, T_a * 128), np.int16)
    idx_b = np.zeros((N_CORES, NWA, T_b * 128), np.int16)
    dstl_arr = np.full((N_CORES, NWA, T, 128), -1.0, np.float16)

    a_m = s_b == 0
    flat = (c_arr[a_m] * NWA + w_arr[a_m]) * (T_a * 128) + rank[a_m]
    idx_a.reshape(-1)[flat] = s_pos[a_m].astype(np.int16)
    t_g = rank[a_m] // 128
    e_g = rank[a_m] % 128
    flat = ((c_arr[a_m] * NWA + w_arr[a_m]) * T + t_g) * 128 + e_g
    dstl_arr.reshape(-1)[flat] = s_dstl[a_m].astype(np.float16)

    b_m = ~a_m
    flat = (c_arr[b_m] * NWA + w_arr[b_m]) * (T_b * 128) + rank[b_m]
    idx_b.reshape(-1)[flat] = s_pos[b_m].astype(np.int16)
    t_g = rank[b_m] // 128 + T_a
    e_g = rank[b_m] % 128
    flat = ((c_arr[b_m] * NWA + w_arr[b_m]) * T + t_g) * 128 + e_g
    dstl_arr.reshape(-1)[flat] = s_dstl[b_m].astype(np.float16)

    def wrap(vals):  # [NW*Tc*128] -> [128, NW*Tc*8] int16 wrapped+replicated
        v = vals.reshape(-1, 16).T
        return np.tile(v, (8, 1)).copy()

    idx_a_w = np.stack([wrap(idx_a[c].reshape(-1)) for c in range(N_CORES)])
    idx_b_w = np.stack([wrap(idx_b[c].reshape(-1)) for c in range(N_CORES)])
    dstloc = np.ascontiguousarray(dstl_arr.transpose(0, 3, 1, 2))  # [C,128,NW,T]
    return idx_a_w, idx_b_w, dstloc, T_a, T_b, perm


def _build(T_a, T_b):
    from concourse import bacc, tile, mybir, library_config

    dt = mybir.dt
    T = T_a + T_b
    nc = bacc.Bacc("TRN2", target_bir_lowering=False, debug=False,
                   num_devices=N_CORES, num_swdge_queues=4,
                   dynamic_dma_scratch_size=49152)

    # ---- I/O ----
    xT_in = nc.dram_tensor("xT", [IN_DIM, NPC], dt.float32, kind="ExternalInput")
    w_inT_in = nc.dram_tensor("w_inT", [IN_DIM, HID], dt.float32,
                              kind="ExternalInput")
    b_in_in = nc.dram_tensor("b_in", [HID, 1], dt.float32, kind="ExternalInput")
    w_lT_in = nc.dram_tensor("w_lT", [N_LAYERS, HID, HID], dt.float32,
                             kind="ExternalInput")
    b_l_in = nc.dram_tensor("b_l", [N_LAYERS, HID, 1], dt.float32,
                            kind="ExternalInput")
    w_out4_in = nc.dram_tensor("w_out4", [N_LAYERS + 1, HID, OUT_DIM],
                               dt.float16, kind="ExternalInput")
    b_out_in = nc.dram_tensor("b_out", [OUT_DIM, 1], dt.float32,
                              kind="ExternalInput")
    iota_in = nc.dram_tensor("iota_wt", [128, AGW, T], dt.float16,
                             kind="ExternalInput")
    id96_in = nc.dram_tensor("id96", [HID, HID], dt.float16,
                             kind="ExternalInput")
    id128_in = nc.dram_tensor("id128", [128, 128], dt.float32,
                              kind="ExternalInput")
    idx_a_in = nc.dram_tensor("idx_a", [128, NWA * T_a * 8], dt.int16,
                              kind="ExternalInput")
    idx_b_in = nc.dram_tensor("idx_b", [128, NWA * T_b * 8], dt.int16,
                              kind="ExternalInput")
    dstloc_in = nc.dram_tensor("dstloc", [128, NWA, T], dt.float16,
                               kind="ExternalInput")
    out_ext = nc.dram_tensor("out", [NPC, OUT_DIM], dt.float32,
                             kind="ExternalOutput")

    f32, f32r, f16 = dt.float32, dt.float32r, dt.float16

    with tile.TileContext(nc, num_cores=N_CORES) as tc:
        nc.gpsimd.load_library(library_config.mlp)
        with tc.tile_pool(name="persist", bufs=1) as pp, \
             tc.tile_pool(name="xpool", bufs=2) as xpool, \
             tc.tile_pool(name="hp", bufs=3) as hp_pool, \
             tc.tile_pool(name="ga", bufs=3) as ga_pool, \
             tc.tile_pool(name="gb", bufs=3) as gb_pool, \
             tc.tile_pool(name="oha", bufs=6) as oha_pool, \
             tc.tile_pool(name="ohb", bufs=6) as ohb_pool, \
             tc.tile_pool(name="otile", bufs=2) as ot_pool, \
             tc.tile_pool(name="ps_agg", bufs=3, space="PSUM") as ps_agg, \
             tc.tile_pool(name="ps_big", bufs=2, space="PSUM") as ps_big, \
             tc.tile_pool(name="ps_tr", bufs=2, space="PSUM") as ps_tr, \
             tc.tile_pool(name="dram", bufs=1, space="DRAM") as dram:

            def load(name, shape, dtype, src_ap):
                t = pp.tile(shape, dtype, name=name)
                nc.sync.dma_start(out=t[:], in_=src_ap)
                return t

            w_inT = load("w_inT", [IN_DIM, HID], f32r, w_inT_in[:].bitcast(f32r))
            b_in = load("b_in", [HID, 1], f32, b_in_in[:])
            w_lT = [load(f"w_lT{l}", [HID, HID], f32r, w_lT_in[l].bitcast(f32r))
                    for l in range(N_LAYERS)]
            b_l = [load(f"b_l{l}", [HID, 1], f32, b_l_in[l])
                   for l in range(N_LAYERS)]
            w_out4 = [load(f"w_out4_{s}", [HID, OUT_DIM], f16, w_out4_in[s])
                      for s in range(N_LAYERS + 1)]
            b_out = load("b_out", [OUT_DIM, 1], f32, b_out_in[:])
            iota_wt = load("iota_wt", [128, AGW, T], f16, iota_in[:])
            id96 = load("id96", [HID, HID], f16, id96_in[:])
            id128 = load("id128", [128, 128], f32, id128_in[:])
            idx_a = load("idx_a", [128, NWA * T_a * 8], dt.int16, idx_a_in[:])
            idx_b = load("idx_b", [128, NWA * T_b * 8], dt.int16, idx_b_in[:])
            dstloc = load("dstloc", [128, NWA, T], f16, dstloc_in[:])

            h_state = [pp.tile([HID, NPC], f16, name=f"h{s}")
                       for s in range(N_LAYERS + 1)]
            rm_buf = pp.tile([128, NW, 128], f16, name="rm_buf")
            # zero the pad columns once so gathered rows are NaN-free and
            # the aggregation matmul can use the full 128-col lhsT (FWL)
            nc.vector.memset(rm_buf[:, :, HID:128], 0.0)

            # double-buffered replicated tables: state s lives in buf s%2
            h_fullA = [dram.tile([CLS, 128], f16, name=f"h_fullA{i}",
                                 addr_space="Shared")
                       for i in range(N_LAYERS)]
            h_fullB = [dram.tile([N_NODES - CLS, 128], f16, name=f"h_fullB{i}",
                                 addr_space="Shared")
                       for i in range(N_LAYERS)]
            bounceA = dram.tile([HALF, 128], f16)
            bounceB = dram.tile([NPC - HALF, 128], f16)

            w_chunks = [(c0, min(CHUNK_W, NW - c0))
                        for c0 in range(0, NW, CHUNK_W)]

            def transpose_windows(s, w0, w1):
                for t in range(w0, w1):
                    n0 = t * 128
                    tn = min(128, NPC - n0)
                    pst = ps_tr.tile([128, HID], f16, name="pst")
                    nc.tensor.transpose(pst[:tn, :],
                                        h_state[s][:, n0:n0 + tn], id96[:])
                    nc.scalar.copy(rm_buf[:tn, t, 0:HID], pst[:tn, :])

            def bounce_chunk(j):
                # incremental bounce: ship chunk j's freshly transposed
                # windows to the DRAM staging buffer right away, so the
                # AllGather emitted later waits ~1us instead of a full
                # half-table DMA (AW is CHUNK_W-aligned: chunks 0..7 are
                # exactly the A half)
                c0, cw = w_chunks[j]
                if c0 < AW:
                    nc.sync.dma_start(
                        out=bounceA[c0 * WIN:(c0 + cw) * WIN, :].rearrange(
                            "(t p) d -> p t d", p=128),
                        in_=rm_buf[:, c0:c0 + cw, :])
                    return
                o0 = (c0 - AW) * WIN
                full = cw if c0 + cw < NW else cw - 1
                if full:
                    nc.sync.dma_start(
                        out=bounceB[o0:o0 + full * WIN, :].rearrange(
                            "(t p) d -> p t d", p=128),
                        in_=rm_buf[:, c0:c0 + full, :])
                if c0 + cw == NW:
                    last_n = NPC - (NW - 1) * WIN
                    o1 = o0 + full * WIN
                    nc.sync.dma_start(out=bounceB[o1:o1 + last_n, :],
                                      in_=rm_buf[0:last_n, NW - 1, :])

            def all_gather_a(s):
                nc.gpsimd.collective_compute(
                    "AllGather", mybir.AluOpType.bypass,
                    ins=[bounceA.opt()], outs=[h_fullA[s].opt()],
                    replica_groups=[list(range(N_CORES))])

            def all_gather_b(s):
                nc.gpsimd.collective_compute(
                    "AllGather", mybir.AluOpType.bypass,
                    ins=[bounceB.opt()], outs=[h_fullB[s].opt()],
                    replica_groups=[list(range(N_CORES))])

            qrr = [0]

            def emit_gathers(gbuf, src_view, idx_tile, base_tile, n_tiles):
                for s0 in range(0, n_tiles, GT):
                    sn = min(GT, n_tiles - s0)
                    nc.gpsimd.dma_gather(
                        gbuf[:, s0:s0 + sn, :], src_view,
                        idx_tile[:, (base_tile + s0) * 8:
                                 (base_tile + s0 + sn) * 8],
                        num_idxs=sn * 128, num_idxs_reg=sn * 128,
                        elem_size=128, single_packet=SINGLE_PACKET,
                        queue_num=qrr[0] % 4)
                    qrr[0] += 1

            # ---- input projection (chunk-pipelined epilogue) ----
            for j, (c0, cw) in enumerate(w_chunks):
                n0, cn = c0 * WIN, min(CW_N, NPC - c0 * WIN)
                xb = xpool.tile([IN_DIM, CW_N], f32r, name="xb")
                nc.sync.dma_start(out=xb[:, :cn],
                                  in_=xT_in[:, n0:n0 + cn].bitcast(f32r))
                ps = ps_big.tile([HID, CW_N], f32, name="psb")
                nc.tensor.matmul(ps[:, :cn], w_inT[:], xb[:, :cn],
                                 start=True, stop=True)
                nc.scalar.add(h_state[0][:, n0:n0 + cn], ps[:, :cn], b_in[:])
                transpose_windows(0, c0, c0 + cw)
                bounce_chunk(j)
                if j == BOUNCE_A_CHUNK:
                    all_gather_a(0)
            all_gather_b(0)

            # ---- GIN layers ----
            for l in range(N_LAYERS):
                tblA = h_fullA[l][:]
                tblB = h_fullB[l][:]
                ga_t, gb_t = {}, {}

                def emit_A(j, tblA=tblA, ga_t=ga_t):
                    c0a = j * APC
                    cwa = min(APC, NWA - c0a)
                    g = ga_pool.tile([128, APC * T_a, 128], f16,
                                     name="g_a")
                    emit_gathers(g, tblA, idx_a, c0a * T_a, cwa * T_a)
                    ga_t[j] = g

                def emit_B(j, tblB=tblB, gb_t=gb_t):
                    c0a = j * APC
                    cwa = min(APC, NWA - c0a)
                    g = gb_pool.tile([128, APC * T_b, 128], f16,
                                     name="g_b")
                    emit_gathers(g, tblB, idx_b, c0a * T_b, cwa * T_b)
                    gb_t[j] = g

                # The PE stream runs the A-class accumulation of chunk j+2
                # ahead of the B-class accumulation of chunk j: each chunk
                # owns a full PSUM bank whose per-window groups stay open
                # (A part stop=False) until the B tiles close them two
                # slots later.  At a layer boundary the PE therefore has
                # ~2 chunks of A work queued before the first instruction
                # that needs the (still in flight) AllGather-B table.
                ps_t = {}

                def a_chains(j, ps_t=ps_t, ga_t=ga_t):
                    c0a = j * APC
                    cwa = min(APC, NWA - c0a)
                    g_a = ga_t.pop(j)
                    ps = ps_agg.tile([128, CW_N], f32, name="psa")
                    ps_t[j] = ps
                    for wl in range(cwa):
                        w_i = c0a + wl
                        oha = oha_pool.tile([128, AGW, T_a], f16, name="oha")
                        nc.vector.tensor_tensor(
                            oha[:],
                            iota_wt[:, :, 0:T_a],
                            dstloc[:, w_i, 0:T_a].unsqueeze(1)
                                .broadcast_to([128, AGW, T_a]),
                            mybir.AluOpType.is_equal)
                        # full 128-col lhsT (pad cols are zeros) -> FWL
                        # fast weight load; only partitions 0:HID are read.
                        # start=True zeroes the WHOLE 2KB bank (zero-region
                        # granularity), so the bank is one accumulation
                        # group: start only on its very first matmul, stop
                        # only on its very last (in b_chains); unwritten
                        # elements first-write via has_written.
                        for t in range(T_a):
                            nc.tensor.matmul(
                                ps[:, wl * AGW:(wl + 1) * AGW],
                                g_a[:, wl * T_a + t, :], oha[:, :, t],
                                start=(wl == 0 and t == 0), stop=False,
                                skip_group_check=True)

                def b_chains(j, ps_t=ps_t, gb_t=gb_t):
                    c0a = j * APC
                    cwa = min(APC, NWA - c0a)
                    g_b = gb_t.pop(j)
                    ps = ps_t[j]
                    for wl in range(cwa):
                        w_i = c0a + wl
                        ohb = ohb_pool.tile([128, AGW, T_b], f16, name="ohb")
                        nc.vector.tensor_tensor(
                            ohb[:],
                            iota_wt[:, :, 0:T_b],
                            dstloc[:, w_i, T_a:T].unsqueeze(1)
                                .broadcast_to([128, AGW, T_b]),
                            mybir.AluOpType.is_equal)
                        for t in range(T_b):
                            nc.tensor.matmul(
                                ps[:, wl * AGW:(wl + 1) * AGW],
                                g_b[:, wl * T_b + t, :], ohb[:, :, t],
                                start=False,
                                stop=(wl == cwa - 1 and t == T_b - 1),
                                skip_group_check=True)

                emit_A(0)
                emit_A(1)
                emit_A(2)
                emit_B(0)
                emit_B(1)
                a_chains(0)
                a_chains(1)
                for j, (c0, cw) in enumerate(w_chunks):
                    if j + 3 < len(w_chunks):
                        emit_A(j + 3)
                    if j + 2 < len(w_chunks):
                        emit_B(j + 2)
                    b_chains(j)
                    # a_chains(j+2) keeps the PE busy while the DVE runs
                    # chunk j's +h add, so the dense matmul's input is
                    # ready the moment the PE reaches it
                    if j + 2 < len(w_chunks):
                        a_chains(j + 2)
                    n0, cn = c0 * WIN, min(CW_N, NPC - c0 * WIN)
                    ps = ps_t.pop(j)
                    hp = hp_pool.tile([HID, CW_N], f32r, name="hp")
                    nc.vector.tensor_tensor(
                        hp[:, :cn], ps[:HID, :cn],
                        h_state[l][:, n0:n0 + cn], mybir.AluOpType.add)
                    ps2 = ps_big.tile([HID, CW_N], f32, name="psb")
                    nc.tensor.matmul(ps2[:, :cn], w_lT[l][:], hp[:, :cn],
                                     start=True, stop=True)
                    nc.scalar.add(h_state[l + 1][:, n0:n0 + cn], ps2[:, :cn],
                                  b_l[l][:])
                    if l < N_LAYERS - 1:
                        transpose_windows(l + 1, c0, c0 + cw)
                        bounce_chunk(j)
                        if j == BOUNCE_A_CHUNK:
                            all_gather_a(l + 1)
                    else:
                        # interleave the output projection into layer 2
                        pso = ps_big.tile([OUT_DIM, CW_N], f32, name="pso",
                                          tag="psb")
                        for s in range(N_LAYERS + 1):
                            nc.tensor.matmul(pso[:, :cn], w_out4[s][:],
                                             h_state[s][:, n0:n0 + cn],
                                             start=(s == 0),
                                             stop=(s == N_LAYERS))
                        ot = ot_pool.tile([OUT_DIM, CW_N], f32, name="ot")
                        nc.scalar.add(ot[:, :cn], pso[:, :cn], b_out[:])
                        for tt in range(-(-cn // 128)):
                            t0 = tt * 128
                            tn = min(128, cn - t0)
                            pst = ps_tr.tile([128, 128], f32, name="psto",
                                             tag="pst")
                            nc.tensor.transpose(pst[:tn, :],
                                                ot[:, t0:t0 + tn], id128[:])
                            orow = ot_pool.tile([128, 128], f32, name="orow")
                            nc.scalar.copy(orow[:tn, :], pst[:tn, :])
                            nc.sync.dma_start(
                                out=out_ext[n0 + t0:n0 + t0 + tn, :],
                                in_=orow[:tn, :])
                if l < N_LAYERS - 1:
                    all_gather_b(l + 1)

    nc.compile()
    return nc


def _get_nc_and_inputs(inputs):
    from concourse import bass_utils  # noqa: F401  (path setup)

    x = np.asarray(inputs["x"], np.float32)
    edge_index = np.asarray(inputs["edge_index"], np.int32)
    W_in = np.asarray(inputs["W_in"], np.float32)
    b_in = np.asarray(inputs["b_in"], np.float32)
    W_layers = np.asarray(inputs["W_layers"], np.float32)
    b_layers = np.asarray(inputs["b_layers"], np.float32)
    W_out = np.asarray(inputs["W_out"], np.float32)
    b_out = np.asarray(inputs["b_out"], np.float32)

    idx_a_w, idx_b_w, dstloc, T_a, T_b, perm = _prep(edge_index)

    key = ("nc", T_a, T_b)
    if key not in _cache:
        _cache.clear()
        _cache[key] = _build(T_a, T_b)
    nc = _cache[key]

    T = T_a + T_b
    inv = np.empty(N_NODES, np.int64)
    inv[perm] = np.arange(N_NODES)
    xT = np.ascontiguousarray(x.T[:, inv])
    w_inT = np.ascontiguousarray(W_in.T)
    w_lT = np.ascontiguousarray(W_layers.transpose(0, 2, 1))
    b_l = np.ascontiguousarray(b_layers[:, :, None])
    w_out4 = np.ascontiguousarray(
        np.stack([W_out[:, s * HID:(s + 1) * HID].T
                  for s in range(N_LAYERS + 1)])).astype(np.float16)
    iota_wt = np.ascontiguousarray(np.broadcast_to(
        np.arange(AGW, dtype=np.float16)[None, :, None],
        (128, AGW, T)))
    id96 = np.eye(HID, dtype=np.float16)
    id128 = np.eye(128, dtype=np.float32)

    in_maps = []
    for c in range(N_CORES):
        in_maps.append({
            "xT": np.ascontiguousarray(xT[:, c * NPC:(c + 1) * NPC]),
            "w_inT": w_inT,
            "b_in": b_in.reshape(HID, 1),
            "w_lT": w_lT,
            "b_l": b_l,
            "w_out4": w_out4,
            "b_out": b_out.reshape(OUT_DIM, 1),
            "iota_wt": iota_wt,
            "id96": id96,
            "id128": id128,
            "idx_a": idx_a_w[c],
            "idx_b": idx_b_w[c],
            "dstloc": dstloc[c],
        })
    return nc, in_maps, perm


def run(inputs, trace=False):
    from concourse import bass_utils

    nc, in_maps, perm = _get_nc_and_inputs(inputs)
    res = bass_utils.run_bass_kernel_spmd(
        nc, in_maps, core_ids=list(range(N_CORES)), trace=trace)
    out = np.concatenate([res.results[c]["out"] for c in range(N_CORES)], 0)
    return out[perm], res


def kernel(**inputs):
    out, _ = run(inputs, trace=False)
    return out



# revision 34
# speedup vs baseline: 1.4942x; 1.0497x over previous
"""GIN-style 3-layer GNN encoder on 8 Trainium2 NeuronCores (Bass/Tile).

Reference computation (fp32):
    h = x @ W_in.T + b_in                                  [50000, 96]
    for l in 0..2:
        agg = segment_sum(h[src], dst, N)                  [50000, 96]
        h = (h + agg) @ W_layers[l].T + b_layers[l]
    out = concat([h0..h3], 1) @ W_out.T + b_out            [50000, 128]

Distribution: nodes are partitioned across the 8 cores (6250/core) via a
host-side balancing permutation; each edge is owned by the core that owns
its dst node.  Each layer the updated node features are AllGathered into
two replicated row-major fp16 tables h_fullA/h_fullB (asymmetric split:
A = first 4096 nodes/core = 32768 rows globally, exactly the int16 index
limit; B = the remaining 2154/core).  The asymmetry lets AllGather-A
fire at ~62% of the layer (right after the A half is transposed and
bounced incrementally per chunk) so its long flight hides under the
rest of the layer, while the small AllGather-B at the layer end lands
quickly.

Per-core segment sum: a core's node range is split into 49 windows of 128
nodes.  Every window has a fixed number of 128-edge tiles (T_a tiles with
src in half A, T_b in half B; the balancing permutation equalizes
per-window per-class edge counts so the fixed tile counts are tight).
Edge features are fetched with gpsimd dma_gather (fp16 256B rows, 1024
idxs per instruction, round-robin over the 4 SWDGE queues).  For each
window the one-hot onehot[e, j, t] = (j == dst_local[e, t]) is built on
DVE (layout [128, WIN, T] keeps every operand's last dim stride-1 so the
2x DVE perf mode engages), and the PE accumulates
    psum[96, 128] += gathered_tile[128e, 96].T @ onehot[:, :, t]
which is aggT for the window.

The whole layer is chunk-pipelined: each chunk of 4 windows (512 nodes =
one full PSUM bank holding one open accumulation group — start only on
the bank's first matmul, stop on its last, since start=True zeroes the
whole 2KB zero-region) flows gathers -> onehot/agg -> +h (DVE, chunk
wide) -> layer matmul -> bias -> PE transpose -> per-chunk bounce DMA.
The A-class chains of chunk j+2 run ahead of the B-class chains of
chunk j in the PE stream, and A gathers are emitted a slot earlier
still, so the AllGather-B flight at each layer boundary is covered by
queued A work.  The aggregation matmuls use the full 128-col gathered
tile as lhsT (pad columns zeroed once in rm_buf) so the compiler's
fast-weight-load kicks in (~1.4x faster LDWEIGHTS).
The final output projection is interleaved into layer 2's chunk loop.
"""
import sys

sys.path.insert(0, "/opt/trn_rl_repo")

import numpy as np

N_NODES = 50000
N_EDGES = 800000
IN_DIM = 128
HID = 96
OUT_DIM = 128
N_LAYERS = 3
N_CORES = 8
NPC = N_NODES // N_CORES          # 6250 nodes per core
WIN = 128                         # window width (nodes)
NW = (NPC + WIN - 1) // WIN       # 49 windows per core (last = 106 nodes)
HALF = 4096                       # per-core A/B split: A = first 4096 nodes
                                  # (global A = 32768 rows = int16 limit);
                                  # asymmetric so AllGather-B is small and
                                  # lands quickly at the layer boundary
BHALF = NPC - HALF                # 2154 B nodes per core
CLS = N_CORES * HALF              # 32768: A-class size
AW = HALF // WIN                  # 32 full-A windows per core
REM_A = HALF - AW * WIN           # 0 A-slots in window AW
CHUNK_W = 4                       # windows per chunk (= 512 nodes = 1 bank)
AGW = 128                         # aggregation window width (PE onehot
                                  # matmul free dim)
NWA = (NPC + AGW - 1) // AGW      # 98 agg windows per core (last = 42)
APC = CHUNK_W * WIN // AGW        # 8 agg windows per chunk
GT = 8                            # tiles per dma_gather (1024 idxs = the
                                  # 64-descs-per-engine single_packet cap)
SINGLE_PACKET = True
CW_N = CHUNK_W * WIN              # 512: node-chunk for dense matmuls
BOUNCE_A_CHUNK = (AW * WIN + REM_A - 1) // CW_N   # chunk whose transposes
                                                  # complete the A half (6)

_cache = {}


def _balance_nodes(src0, dst0):
    """Permute node ids so per-(core,window) A/B edge counts are even.

    A node's A/B class (which replicated gather table its row lives in) is
    frozen to its OLD id (< CLS -> A); the permutation only moves nodes
    within their class region, so per-node (deg_a, deg_b) are fixed and a
    greedy 2-D balance over the 392 (core, window) bins makes the uniform
    tile counts T_a/T_b tight.  Returns perm (old id -> new id).
    """
    deg_a = np.bincount(dst0[src0 < CLS], minlength=N_NODES).astype(np.int64)
    deg_b = np.bincount(dst0[src0 >= CLS], minlength=N_NODES).astype(np.int64)
    nbins = N_CORES * NWA
    base = np.empty(nbins, np.int64)
    cap = np.empty(nbins, np.int64)
    for b in range(nbins):
        c, w = divmod(b, NWA)
        base[b] = c * NPC + w * AGW
        cap[b] = min(AGW, NPC - w * AGW)
    woff = base % NPC
    q_a = np.clip(HALF - woff, 0, cap)   # A slots = first q_a of the window
    q_b = cap - q_a

    mu_a = max(1.0, deg_a.sum() / nbins)
    mu_b = max(1.0, deg_b.sum() / nbins)
    order = np.argsort(-(deg_a + deg_b), kind="stable")
    a_load = np.zeros(nbins)
    b_load = np.zeros(nbins)
    a_left = q_a.copy()
    b_left = q_b.copy()
    a_pos = np.zeros(nbins, np.int64)
    b_pos = q_a.copy()
    perm = np.empty(N_NODES, np.int64)
    for n in order:
        phi = np.maximum((a_load + deg_a[n]) / mu_a,
                         (b_load + deg_b[n]) / mu_b)
        if n < CLS:
            phi = np.where(a_left > 0, phi, np.inf)
            b_ = int(np.argmin(phi))
            perm[n] = base[b_] + a_pos[b_]
            a_pos[b_] += 1
            a_left[b_] -= 1
        else:
            phi = np.where(b_left > 0, phi, np.inf)
            b_ = int(np.argmin(phi))
            perm[n] = base[b_] + b_pos[b_]
            b_pos[b_] += 1
            b_left[b_] -= 1
        a_load[b_] += deg_a[n]
        b_load[b_] += deg_b[n]
    return perm


def _prep(edge_index):
    """Host-side edge bucketing -> per-core gather index / dst tables."""
    src0 = edge_index[0].astype(np.int64)
    dst0 = edge_index[1].astype(np.int64)
    perm = _balance_nodes(src0, dst0)
    src = perm[src0]
    dst = perm[dst0]
    core = dst // NPC
    din = dst % NPC
    w = din // AGW
    dstl = din % AGW
    s_in = src % NPC
    c_src = src // NPC
    is_b = (s_in >= HALF).astype(np.int64)
    pos = np.where(is_b == 0, c_src * HALF + s_in,
                   c_src * BHALF + s_in - HALF)  # < 32768, int16-safe

    key = (core * NWA + w) * 2 + is_b
    order = np.argsort(key, kind="stable")
    s_pos = pos[order]
    s_dstl = dstl[order]
    s_key = key[order]
    s_b = is_b[order]

    counts = np.bincount(key, minlength=N_CORES * NWA * 2)
    T_a = max(1, int(-(-counts.reshape(-1, 2)[:, 0].max() // 128)))
    T_b = max(1, int(-(-counts.reshape(-1, 2)[:, 1].max() // 128)))
    T = T_a + T_b

    starts = np.zeros(N_CORES * NWA * 2, np.int64)
    starts[1:] = np.cumsum(counts)[:-1]
    rank = np.arange(len(s_key)) - starts[s_key]

    c_arr = s_key // (2 * NWA)
    w_arr = (s_key // 2) % NWA

    idx_a = np.zeros((N_CORES, NWA, T_a * 128), np.int16)
    idx_b = np.zeros((N_CORES, NWA, T_b * 128), np.int16)
    dstl_arr = np.full((N_CORES, NWA, T, 128), -1.0, np.float16)

    a_m = s_b == 0
    flat = (c_arr[a_m] * NWA + w_arr[a_m]) * (T_a * 128) + rank[a_m]
    idx_a.reshape(-1)[flat] = s_pos[a_m].astype(np.int16)
    t_g = rank[a_m] // 128
    e_g = rank[a_m] % 128
    flat = ((c_arr[a_m] * NWA + w_arr[a_m]) * T + t_g) * 128 + e_g
    dstl_arr.reshape(-1)[flat] = s_dstl[a_m].astype(np.float16)

    b_m = ~a_m
    flat = (c_arr[b_m] * NWA + w_arr[b_m]) * (T_b * 128) + rank[b_m]
    idx_b.reshape(-1)[flat] = s_pos[b_m].astype(np.int16)
    t_g = rank[b_m] // 128 + T_a
    e_g = rank[b_m] % 128
    flat = ((c_arr[b_m] * NWA + w_arr[b_m]) * T + t_g) * 128 + e_g
    dstl_arr.reshape(-1)[flat] = s_dstl[b_m].astype(np.float16)

    def wrap(vals):  # [NW*Tc*128] -> [128, NW*Tc*8] int16 wrapped+replicated
        v = vals.reshape(-1, 16).T
        return np.tile(v, (8, 1)).copy()

    idx_a_w = np.stack([wrap(idx_a[c].reshape(-1)) for c in range(N_CORES)])
    idx_b_w = np.stack([wrap(idx_b[c].reshape(-1)) for c in range(N_CORES)])
    dstloc = np.ascontiguousarray(dstl_arr.transpose(0, 3, 1, 2))  # [C,128,NW,T]
    return idx_a_w, idx_b_w, dstloc, T_a, T_b, perm


def _build(T_a, T_b):
    from concourse import bacc, tile, mybir, library_config

    dt = mybir.dt
    T = T_a + T_b
    nc = bacc.Bacc("TRN2", target_bir_lowering=False, debug=False,
                   num_devices=N_CORES, num_swdge_queues=4,
                   dynamic_dma_scratch_size=49152)

    # ---- I/O ----
    xT_in = nc.dram_tensor("xT", [IN_DIM, NPC], dt.float32, kind="ExternalInput")
    w_inT_in = nc.dram_tensor("w_inT", [IN_DIM, HID], dt.float32,
                              kind="ExternalInput")
    b_in_in = nc.dram_tensor("b_in", [HID, 1], dt.float32, kind="ExternalInput")
    w_lT_in = nc.dram_tensor("w_lT", [N_LAYERS, HID, HID], dt.float32,
                             kind="ExternalInput")
    b_l_in = nc.dram_tensor("b_l", [N_LAYERS, HID, 1], dt.float32,
                            kind="ExternalInput")
    w_out4_in = nc.dram_tensor("w_out4", [N_LAYERS + 1, HID, OUT_DIM],
                               dt.float16, kind="ExternalInput")
    b_out_in = nc.dram_tensor("b_out", [OUT_DIM, 1], dt.float32,
                              kind="ExternalInput")
    iota_in = nc.dram_tensor("iota_wt", [128, AGW, T], dt.float16,
                             kind="ExternalInput")
    id96_in = nc.dram_tensor("id96", [HID, HID], dt.float16,
                             kind="ExternalInput")
    id128_in = nc.dram_tensor("id128", [128, 128], dt.float32,
                              kind="ExternalInput")
    idx_a_in = nc.dram_tensor("idx_a", [128, NWA * T_a * 8], dt.int16,
                              kind="ExternalInput")
    idx_b_in = nc.dram_tensor("idx_b", [128, NWA * T_b * 8], dt.int16,
                              kind="ExternalInput")
    dstloc_in = nc.dram_tensor("dstloc", [128, NWA, T], dt.float16,
                               kind="ExternalInput")
    out_ext = nc.dram_tensor("out", [NPC, OUT_DIM], dt.float32,
                             kind="ExternalOutput")

    f32, f32r, f16 = dt.float32, dt.float32r, dt.float16

    with tile.TileContext(nc, num_cores=N_CORES) as tc:
        nc.gpsimd.load_library(library_config.mlp)
        with tc.tile_pool(name="persist", bufs=1) as pp, \
             tc.tile_pool(name="xpool", bufs=2) as xpool, \
             tc.tile_pool(name="hp", bufs=3) as hp_pool, \
             tc.tile_pool(name="ga", bufs=3) as ga_pool, \
             tc.tile_pool(name="gb", bufs=3) as gb_pool, \
             tc.tile_pool(name="oha", bufs=6) as oha_pool, \
             tc.tile_pool(name="ohb", bufs=6) as ohb_pool, \
             tc.tile_pool(name="otile", bufs=2) as ot_pool, \
             tc.tile_pool(name="ps_agg", bufs=3, space="PSUM") as ps_agg, \
             tc.tile_pool(name="ps_big", bufs=2, space="PSUM") as ps_big, \
             tc.tile_pool(name="ps_tr", bufs=2, space="PSUM") as ps_tr, \
             tc.tile_pool(name="dram", bufs=1, space="DRAM") as dram:

            def load(name, shape, dtype, src_ap):
                t = pp.tile(shape, dtype, name=name)
                nc.sync.dma_start(out=t[:], in_=src_ap)
                return t

            w_inT = load("w_inT", [IN_DIM, HID], f32r, w_inT_in[:].bitcast(f32r))
            b_in = load("b_in", [HID, 1], f32, b_in_in[:])
            w_lT = [load(f"w_lT{l}", [HID, HID], f32r, w_lT_in[l].bitcast(f32r))
                    for l in range(N_LAYERS)]
            b_l = [load(f"b_l{l}", [HID, 1], f32, b_l_in[l])
                   for l in range(N_LAYERS)]
            w_out4 = [load(f"w_out4_{s}", [HID, OUT_DIM], f16, w_out4_in[s])
                      for s in range(N_LAYERS + 1)]
            b_out = load("b_out", [OUT_DIM, 1], f32, b_out_in[:])
            iota_wt = load("iota_wt", [128, AGW, T], f16, iota_in[:])
            id96 = load("id96", [HID, HID], f16, id96_in[:])
            id128 = load("id128", [128, 128], f32, id128_in[:])
            idx_a = load("idx_a", [128, NWA * T_a * 8], dt.int16, idx_a_in[:])
            idx_b = load("idx_b", [128, NWA * T_b * 8], dt.int16, idx_b_in[:])
            dstloc = load("dstloc", [128, NWA, T], f16, dstloc_in[:])

            h_state = [pp.tile([HID, NPC], f16, name=f"h{s}")
                       for s in range(N_LAYERS + 1)]
            rm_buf = pp.tile([128, NW, 128], f16, name="rm_buf")
            # zero the pad columns once so gathered rows are NaN-free and
            # the aggregation matmul can use the full 128-col lhsT (FWL)
            nc.vector.memset(rm_buf[:, :, HID:128], 0.0)

            # double-buffered replicated tables: state s lives in buf s%2
            h_fullA = [dram.tile([CLS, 128], f16, name=f"h_fullA{i}",
                                 addr_space="Shared")
                       for i in range(N_LAYERS)]
            h_fullB = [dram.tile([N_NODES - CLS, 128], f16, name=f"h_fullB{i}",
                                 addr_space="Shared")
                       for i in range(N_LAYERS)]
            bounceA = dram.tile([HALF, 128], f16)
            bounceB = dram.tile([NPC - HALF, 128], f16)

            w_chunks = [(c0, min(CHUNK_W, NW - c0))
                        for c0 in range(0, NW, CHUNK_W)]

            def transpose_windows(s, w0, w1):
                for t in range(w0, w1):
                    n0 = t * 128
                    tn = min(128, NPC - n0)
                    pst = ps_tr.tile([128, HID], f16, name="pst")
                    nc.tensor.transpose(pst[:tn, :],
                                        h_state[s][:, n0:n0 + tn], id96[:])
                    nc.scalar.copy(rm_buf[:tn, t, 0:HID], pst[:tn, :])

            def bounce_chunk(j):
                # incremental bounce: ship chunk j's freshly transposed
                # windows to the DRAM staging buffer right away, so the
                # AllGather emitted later waits ~1us instead of a full
                # half-table DMA (AW is CHUNK_W-aligned: chunks 0..7 are
                # exactly the A half)
                c0, cw = w_chunks[j]
                if c0 < AW:
                    nc.sync.dma_start(
                        out=bounceA[c0 * WIN:(c0 + cw) * WIN, :].rearrange(
                            "(t p) d -> p t d", p=128),
                        in_=rm_buf[:, c0:c0 + cw, :])
                    return
                o0 = (c0 - AW) * WIN
                full = cw if c0 + cw < NW else cw - 1
                if full:
                    nc.sync.dma_start(
                        out=bounceB[o0:o0 + full * WIN, :].rearrange(
                            "(t p) d -> p t d", p=128),
                        in_=rm_buf[:, c0:c0 + full, :])
                if c0 + cw == NW:
                    last_n = NPC - (NW - 1) * WIN
                    o1 = o0 + full * WIN
                    nc.sync.dma_start(out=bounceB[o1:o1 + last_n, :],
                                      in_=rm_buf[0:last_n, NW - 1, :])

            def all_gather_a(s):
                nc.gpsimd.collective_compute(
                    "AllGather", mybir.AluOpType.bypass,
                    ins=[bounceA.opt()], outs=[h_fullA[s].opt()],
                    replica_groups=[list(range(N_CORES))])

            def all_gather_b(s):
                nc.gpsimd.collective_compute(
                    "AllGather", mybir.AluOpType.bypass,
                    ins=[bounceB.opt()], outs=[h_fullB[s].opt()],
                    replica_groups=[list(range(N_CORES))])

            qrr = [0]

            def emit_gathers(gbuf, src_view, idx_tile, base_tile, n_tiles):
                for s0 in range(0, n_tiles, GT):
                    sn = min(GT, n_tiles - s0)
                    nc.gpsimd.dma_gather(
                        gbuf[:, s0:s0 + sn, :], src_view,
                        idx_tile[:, (base_tile + s0) * 8:
                                 (base_tile + s0 + sn) * 8],
                        num_idxs=sn * 128, num_idxs_reg=sn * 128,
                        elem_size=128, single_packet=SINGLE_PACKET,
                        queue_num=qrr[0] % 4)
                    qrr[0] += 1

            # ---- input projection (chunk-pipelined epilogue) ----
            for j, (c0, cw) in enumerate(w_chunks):
                n0, cn = c0 * WIN, min(CW_N, NPC - c0 * WIN)
                xb = xpool.tile([IN_DIM, CW_N], f32r, name="xb")
                nc.sync.dma_start(out=xb[:, :cn],
                                  in_=xT_in[:, n0:n0 + cn].bitcast(f32r))
                ps = ps_big.tile([HID, CW_N], f32, name="psb")
                nc.tensor.matmul(ps[:, :cn], w_inT[:], xb[:, :cn],
                                 start=True, stop=True)
                nc.scalar.add(h_state[0][:, n0:n0 + cn], ps[:, :cn], b_in[:])
                transpose_windows(0, c0, c0 + cw)
                bounce_chunk(j)
                if j == BOUNCE_A_CHUNK:
                    all_gather_a(0)
            all_gather_b(0)

            # ---- GIN layers ----
            for l in range(N_LAYERS):
                tblA = h_fullA[l][:]
                tblB = h_fullB[l][:]
                ga_t, gb_t = {}, {}

                def emit_A(j, tblA=tblA, ga_t=ga_t):
                    c0a = j * APC
                    cwa = min(APC, NWA - c0a)
                    g = ga_pool.tile([128, APC * T_a, 128], f16,
                                     name="g_a")
                    emit_gathers(g, tblA, idx_a, c0a * T_a, cwa * T_a)
                    ga_t[j] = g

                def emit_B(j, tblB=tblB, gb_t=gb_t):
                    c0a = j * APC
                    cwa = min(APC, NWA - c0a)
                    g = gb_pool.tile([128, APC * T_b, 128], f16,
                                     name="g_b")
                    emit_gathers(g, tblB, idx_b, c0a * T_b, cwa * T_b)
                    gb_t[j] = g

                # The PE stream runs the A-class accumulation of chunk j+2
                # ahead of the B-class accumulation of chunk j: each chunk
                # owns a full PSUM bank whose per-window groups stay open
                # (A part stop=False) until the B tiles close them two
                # slots later.  At a layer boundary the PE therefore has
                # ~2 chunks of A work queued before the first instruction
                # that needs the (still in flight) AllGather-B table.
                ps_t = {}

                def a_chains(j, ps_t=ps_t, ga_t=ga_t):
                    c0a = j * APC
                    cwa = min(APC, NWA - c0a)
                    g_a = ga_t.pop(j)
                    ps = ps_agg.tile([128, CW_N], f32, name="psa")
                    ps_t[j] = ps
                    for wl in range(cwa):
                        w_i = c0a + wl
                        oha = oha_pool.tile([128, AGW, T_a], f16, name="oha")
                        nc.vector.tensor_tensor(
                            oha[:],
                            iota_wt[:, :, 0:T_a],
                            dstloc[:, w_i, 0:T_a].unsqueeze(1)
                                .broadcast_to([128, AGW, T_a]),
                            mybir.AluOpType.is_equal)
                        # full 128-col lhsT (pad cols are zeros) -> FWL
                        # fast weight load; only partitions 0:HID are read.
                        # start=True zeroes the WHOLE 2KB bank (zero-region
                        # granularity), so the bank is one accumulation
                        # group: start only on its very first matmul, stop
                        # only on its very last (in b_chains); unwritten
                        # elements first-write via has_written.
                        for t in range(T_a):
                            nc.tensor.matmul(
                                ps[:, wl * AGW:(wl + 1) * AGW],
                                g_a[:, wl * T_a + t, :], oha[:, :, t],
                                start=(wl == 0 and t == 0), stop=False,
                                skip_group_check=True)

                def b_chains(j, ps_t=ps_t, gb_t=gb_t):
                    c0a = j * APC
                    cwa = min(APC, NWA - c0a)
                    g_b = gb_t.pop(j)
                    ps = ps_t[j]
                    for wl in range(cwa):
                        w_i = c0a + wl
                        ohb = ohb_pool.tile([128, AGW, T_b], f16, name="ohb")
                        nc.vector.tensor_tensor(
                            ohb[:],
                            iota_wt[:, :, 0:T_b],
                            dstloc[:, w_i, T_a:T].unsqueeze(1)
                                .broadcast_to([128, AGW, T_b]),
                            mybir.AluOpType.is_equal)
                        for t in range(T_b):
                            nc.tensor.matmul(
                                ps[:, wl * AGW:(wl + 1) * AGW],
                                g_b[:, wl * T_b + t, :], ohb[:, :, t],
                                start=False,
                                stop=(wl == cwa - 1 and t == T_b - 1),
                                skip_group_check=True)

                emit_A(0)
                emit_A(1)
                emit_A(2)
                emit_B(0)
                emit_B(1)
                a_chains(0)
                a_chains(1)
                for j, (c0, cw) in enumerate(w_chunks):
                    if j + 3 < len(w_chunks):
                        emit_A(j + 3)
                    if j + 2 < len(w_chunks):
                        emit_B(j + 2)
                    b_chains(j)
                    # a_chains(j+2) keeps the PE busy while the DVE runs
                    # chunk j's +h add, so the dense matmul's input is
                    # ready the moment the PE reaches it
                    if j + 2 < len(w_chunks):
                        a_chains(j + 2)
                    n0, cn = c0 * WIN, min(CW_N, NPC - c0 * WIN)
                    ps = ps_t.pop(j)
                    hp = hp_pool.tile([HID, CW_N], f32r, name="hp")
                    nc.vector.tensor_tensor(
                        hp[:, :cn], ps[:HID, :cn],
                        h_state[l][:, n0:n0 + cn], mybir.AluOpType.add)
                    ps2 = ps_big.tile([HID, CW_N], f32, name="psb")
                    nc.tensor.matmul(ps2[:, :cn], w_lT[l][:], hp[:, :cn],
                                     start=True, stop=True)
                    nc.scalar.add(h_state[l + 1][:, n0:n0 + cn], ps2[:, :cn],
                                  b_l[l][:])
                    if l < N_LAYERS - 1:
                        transpose_windows(l + 1, c0, c0 + cw)
                        bounce_chunk(j)
                        if j == BOUNCE_A_CHUNK:
                            all_gather_a(l + 1)
                    else:
                        # interleave the output projection into layer 2
                        pso = ps_big.tile([OUT_DIM, CW_N], f32, name="pso",
                                          tag="psb")
                        for s in range(N_LAYERS + 1):
                            nc.tensor.matmul(pso[:, :cn], w_out4[s][:],
                                             h_state[s][:, n0:n0 + cn],
                                             start=(s == 0),
                                             stop=(s == N_LAYERS))
                        ot = ot_pool.tile([OUT_DIM, CW_N], f32, name="ot")
                        nc.scalar.add(ot[:, :cn], pso[:, :cn], b_out[:])
                        for tt in range(-(-cn // 128)):
                            t0 = tt * 128
                            tn = min(128, cn - t0)
                            pst = ps_tr.tile([128, 128], f32, name="psto",
                                             tag="pst")
                            nc.tensor.transpose(pst[:tn, :],
                                                ot[:, t0:t0 + tn], id128[:])
                            orow = ot_pool.tile([128, 128], f32, name="orow")
                            nc.scalar.copy(orow[:tn, :], pst[:tn, :])
                            nc.sync.dma_start(
                                out=out_ext[n0 + t0:n0 + t0 + tn, :],
                                in_=orow[:tn, :])
                if l < N_LAYERS - 1:
                    all_gather_b(l + 1)

    nc.compile()
    return nc


def _get_nc_and_inputs(inputs):
    from concourse import bass_utils  # noqa: F401  (path setup)

    x = np.asarray(inputs["x"], np.float32)
    edge_index = np.asarray(inputs["edge_index"], np.int32)
    W_in = np.asarray(inputs["W_in"], np.float32)
    b_in = np.asarray(inputs["b_in"], np.float32)
    W_layers = np.asarray(inputs["W_layers"], np.float32)
    b_layers = np.asarray(inputs["b_layers"], np.float32)
    W_out = np.asarray(inputs["W_out"], np.float32)
    b_out = np.asarray(inputs["b_out"], np.float32)

    idx_a_w, idx_b_w, dstloc, T_a, T_b, perm = _prep(edge_index)

    key = ("nc", T_a, T_b)
    if key not in _cache:
        _cache.clear()
        _cache[key] = _build(T_a, T_b)
    nc = _cache[key]

    T = T_a + T_b
    inv = np.empty(N_NODES, np.int64)
    inv[perm] = np.arange(N_NODES)
    xT = np.ascontiguousarray(x.T[:, inv])
    w_inT = np.ascontiguousarray(W_in.T)
    w_lT = np.ascontiguousarray(W_layers.transpose(0, 2, 1))
    b_l = np.ascontiguousarray(b_layers[:, :, None])
    w_out4 = np.ascontiguousarray(
        np.stack([W_out[:, s * HID:(s + 1) * HID].T
                  for s in range(N_LAYERS + 1)])).astype(np.float16)
    iota_wt = np.ascontiguousarray(np.broadcast_to(
        np.arange(AGW, dtype=np.float16)[None, :, None],
        (128, AGW, T)))
    id96 = np.eye(HID, dtype=np.float16)
    id128 = np.eye(128, dtype=np.float32)

    in_maps = []
    for c in range(N_CORES):
        in_maps.append({
            "xT": np.ascontiguousarray(xT[:, c * NPC:(c + 1) * NPC]),
            "w_inT": w_inT,
            "b_in": b_in.reshape(HID, 1),
            "w_lT": w_lT,
            "b_l": b_l,
            "w_out4": w_out4,
            "b_out": b_out.reshape(OUT_DIM, 1),
            "iota_wt": iota_wt,
            "id96": id96,
            "id128": id128,
            "idx_a": idx_a_w[c],
            "idx_b": idx_b_w[c],
            "dstloc": dstloc[c],
        })
    return nc, in_maps, perm


def run(inputs, trace=False):
    from concourse import bass_utils

    nc, in_maps, perm = _get_nc_and_inputs(inputs)
    res = bass_utils.run_bass_kernel_spmd(
        nc, in_maps, core_ids=list(range(N_CORES)), trace=trace)
    out = np.concatenate([res.results[c]["out"] for c in range(N_CORES)], 0)
    return out[perm], res


def kernel(**inputs):
    out, _ = run(inputs, trace=False)
    return out



# revision 37
# speedup vs baseline: 1.7236x; 1.1535x over previous
"""GIN-style 3-layer GNN encoder on 8 Trainium2 NeuronCores (Bass/Tile).

Reference computation (fp32):
    h = x @ W_in.T + b_in                                  [50000, 96]
    for l in 0..2:
        agg = segment_sum(h[src], dst, N)                  [50000, 96]
        h = (h + agg) @ W_layers[l].T + b_layers[l]
    out = concat([h0..h3], 1) @ W_out.T + b_out            [50000, 128]

Distribution: nodes are partitioned across the 8 cores (6250/core) via a
host-side balancing permutation; each edge is owned by the core that owns
its dst node.  Each layer the updated node features are AllGathered into
two replicated row-major fp16 tables h_fullA/h_fullB (asymmetric split:
A = first 4096 nodes/core = 32768 rows globally, exactly the int16 index
limit; B = the remaining 2154/core).  The asymmetry lets AllGather-A
fire at ~62% of the layer (right after the A half is transposed and
bounced incrementally per chunk) so its long flight hides under the
rest of the layer, while the small AllGather-B at the layer end lands
quickly.

Per-core segment sum: a core's node range is split into 49 windows of 128
nodes.  Every window has a fixed number of 128-edge tiles (T_a tiles with
src in half A, T_b in half B; the balancing permutation equalizes
per-window per-class edge counts so the fixed tile counts are tight).
Edge features are fetched with gpsimd dma_gather (fp16 256B rows, 1024
idxs per instruction, round-robin over the 4 SWDGE queues).  For each
window the one-hot onehot[e, j, t] = (j == dst_local[e, t]) is built on
DVE (layout [128, WIN, T] keeps every operand's last dim stride-1 so the
2x DVE perf mode engages), and the PE accumulates
    psum[96, 128] += gathered_tile[128e, 96].T @ onehot[:, :, t]
which is aggT for the window.

The whole layer is chunk-pipelined: each chunk of 4 windows (512 nodes =
one full PSUM bank holding one open accumulation group — start only on
the bank's first matmul, stop on its last, since start=True zeroes the
whole 2KB zero-region) flows gathers -> onehot/agg -> +h (DVE, chunk
wide) -> layer matmul -> bias -> PE transpose -> per-chunk bounce DMA.
The A-class chains of chunk j+2 run ahead of the B-class chains of
chunk j in the PE stream, and A gathers are emitted a slot earlier
still, so the AllGather-B flight at each layer boundary is covered by
queued A work.  The aggregation matmuls use the full 128-col gathered
tile as lhsT (pad columns zeroed once in rm_buf) so the compiler's
fast-weight-load kicks in (~1.4x faster LDWEIGHTS).
The final output projection is interleaved into layer 2's chunk loop.
"""
import sys

sys.path.insert(0, "/opt/trn_rl_repo")

import numpy as np

N_NODES = 50000
N_EDGES = 800000
IN_DIM = 128
HID = 96
OUT_DIM = 128
N_LAYERS = 3
N_CORES = 8
NPC = N_NODES // N_CORES          # 6250 nodes per core
WIN = 128                         # window width (nodes)
NW = (NPC + WIN - 1) // WIN       # 49 windows per core (last = 106 nodes)
HALF = 4096                       # per-core A/B split: A = first 4096 nodes
                                  # (global A = 32768 rows = int16 limit);
                                  # asymmetric so AllGather-B is small and
                                  # lands quickly at the layer boundary
BHALF = NPC - HALF                # 2154 B nodes per core
CLS = N_CORES * HALF              # 32768: A-class size
AW = HALF // WIN                  # 32 full-A windows per core
REM_A = HALF - AW * WIN           # 0 A-slots in window AW
CHUNK_W = 4                       # windows per chunk (= 512 nodes = 1 bank)
AGW = 128                         # aggregation window width (PE onehot
                                  # matmul free dim)
NWA = (NPC + AGW - 1) // AGW      # 98 agg windows per core (last = 42)
APC = CHUNK_W * WIN // AGW        # 8 agg windows per chunk
GT = 8                            # tiles per dma_gather (1024 idxs = the
                                  # 64-descs-per-engine single_packet cap)
SINGLE_PACKET = True
CW_N = CHUNK_W * WIN              # 512: node-chunk for dense matmuls
BOUNCE_A_CHUNK = (AW * WIN + REM_A - 1) // CW_N   # chunk whose transposes
                                                  # complete the A half (6)

_cache = {}


def _balance_nodes(src0, dst0):
    """Permute node ids so per-(core,window) A/B edge counts are even.

    A node's A/B class (which replicated gather table its row lives in) is
    frozen to its OLD id (< CLS -> A); the permutation only moves nodes
    within their class region, so per-node (deg_a, deg_b) are fixed and a
    greedy 2-D balance over the 392 (core, window) bins makes the uniform
    tile counts T_a/T_b tight.  Returns perm (old id -> new id).
    """
    deg_a = np.bincount(dst0[src0 < CLS], minlength=N_NODES).astype(np.int64)
    deg_b = np.bincount(dst0[src0 >= CLS], minlength=N_NODES).astype(np.int64)
    nbins = N_CORES * NWA
    base = np.empty(nbins, np.int64)
    cap = np.empty(nbins, np.int64)
    for b in range(nbins):
        c, w = divmod(b, NWA)
        base[b] = c * NPC + w * AGW
        cap[b] = min(AGW, NPC - w * AGW)
    woff = base % NPC
    q_a = np.clip(HALF - woff, 0, cap)   # A slots = first q_a of the window
    q_b = cap - q_a

    mu_a = max(1.0, deg_a.sum() / nbins)
    mu_b = max(1.0, deg_b.sum() / nbins)
    order = np.argsort(-(deg_a + deg_b), kind="stable")
    a_load = np.zeros(nbins)
    b_load = np.zeros(nbins)
    a_left = q_a.copy()
    b_left = q_b.copy()
    a_pos = np.zeros(nbins, np.int64)
    b_pos = q_a.copy()
    perm = np.empty(N_NODES, np.int64)
    for n in order:
        phi = np.maximum((a_load + deg_a[n]) / mu_a,
                         (b_load + deg_b[n]) / mu_b)
        if n < CLS:
            phi = np.where(a_left > 0, phi, np.inf)
            b_ = int(np.argmin(phi))
            perm[n] = base[b_] + a_pos[b_]
            a_pos[b_] += 1
            a_left[b_] -= 1
        else:
            phi = np.where(b_left > 0, phi, np.inf)
            b_ = int(np.argmin(phi))
            perm[n] = base[b_] + b_pos[b_]
            b_pos[b_] += 1
            b_left[b_] -= 1
        a_load[b_] += deg_a[n]
        b_load[b_] += deg_b[n]
    return perm


def _prep(edge_index):
    """Host-side edge bucketing -> per-core gather index / dst tables."""
    src0 = edge_index[0].astype(np.int64)
    dst0 = edge_index[1].astype(np.int64)
    perm = _balance_nodes(src0, dst0)
    src = perm[src0]
    dst = perm[dst0]
    core = dst // NPC
    din = dst % NPC
    w = din // AGW
    dstl = din % AGW
    s_in = src % NPC
    c_src = src // NPC
    is_b = (s_in >= HALF).astype(np.int64)
    pos = np.where(is_b == 0, c_src * HALF + s_in,
                   c_src * BHALF + s_in - HALF)  # < 32768, int16-safe

    key = (core * NWA + w) * 2 + is_b
    order = np.argsort(key, kind="stable")
    s_pos = pos[order]
    s_dstl = dstl[order]
    s_key = key[order]
    s_b = is_b[order]

    counts = np.bincount(key, minlength=N_CORES * NWA * 2)
    T_a = max(1, int(-(-counts.reshape(-1, 2)[:, 0].max() // 128)))
    T_b = max(1, int(-(-counts.reshape(-1, 2)[:, 1].max() // 128)))
    T = T_a + T_b

    starts = np.zeros(N_CORES * NWA * 2, np.int64)
    starts[1:] = np.cumsum(counts)[:-1]
    rank = np.arange(len(s_key)) - starts[s_key]

    c_arr = s_key // (2 * NWA)
    w_arr = (s_key // 2) % NWA

    idx_a = np.zeros((N_CORES, NWA, T_a * 128), np.int16)
    idx_b = np.zeros((N_CORES, NWA, T_b * 128), np.int16)
    dstl_arr = np.full((N_CORES, NWA, T, 128), -1.0, np.float16)

    a_m = s_b == 0
    flat = (c_arr[a_m] * NWA + w_arr[a_m]) * (T_a * 128) + rank[a_m]
    idx_a.reshape(-1)[flat] = s_pos[a_m].astype(np.int16)
    t_g = rank[a_m] // 128
    e_g = rank[a_m] % 128
    flat = ((c_arr[a_m] * NWA + w_arr[a_m]) * T + t_g) * 128 + e_g
    dstl_arr.reshape(-1)[flat] = s_dstl[a_m].astype(np.float16)

    b_m = ~a_m
    flat = (c_arr[b_m] * NWA + w_arr[b_m]) * (T_b * 128) + rank[b_m]
    idx_b.reshape(-1)[flat] = s_pos[b_m].astype(np.int16)
    t_g = rank[b_m] // 128 + T_a
    e_g = rank[b_m] % 128
    flat = ((c_arr[b_m] * NWA + w_arr[b_m]) * T + t_g) * 128 + e_g
    dstl_arr.reshape(-1)[flat] = s_dstl[b_m].astype(np.float16)

    def wrap(vals):  # [NW*Tc*128] -> [128, NW*Tc*8] int16 wrapped+replicated
        v = vals.reshape(-1, 16).T
        return np.tile(v, (8, 1)).copy()

    idx_a_w = np.stack([wrap(idx_a[c].reshape(-1)) for c in range(N_CORES)])
    idx_b_w = np.stack([wrap(idx_b[c].reshape(-1)) for c in range(N_CORES)])
    dstloc = np.ascontiguousarray(dstl_arr.transpose(0, 3, 1, 2))  # [C,128,NW,T]
    return idx_a_w, idx_b_w, dstloc, T_a, T_b, perm


def _build(T_a, T_b):
    from concourse import bacc, tile, mybir, library_config

    dt = mybir.dt
    T = T_a + T_b
    nc = bacc.Bacc("TRN2", target_bir_lowering=False, debug=False,
                   num_devices=N_CORES, num_swdge_queues=4,
                   dynamic_dma_scratch_size=49152)

    # ---- I/O ----
    xT_in = nc.dram_tensor("xT", [IN_DIM, NPC], dt.float32, kind="ExternalInput")
    # full replicated x tables (row-major fp16, +c folded in): layer 0
    # gathers raw x instead of h0, so no initial AllGather is needed
    xrA_in = nc.dram_tensor("xrA", [CLS, 128], dt.float16,
                            kind="ExternalInput")
    xrB_in = nc.dram_tensor("xrB", [N_NODES - CLS, 128], dt.float16,
                            kind="ExternalInput")
    wp_in = nc.dram_tensor("wp", [IN_DIM, HID], dt.float32,
                           kind="ExternalInput")
    w_inT_in = nc.dram_tensor("w_inT", [IN_DIM, HID], dt.float32,
                              kind="ExternalInput")
    b_in_in = nc.dram_tensor("b_in", [HID, 1], dt.float32, kind="ExternalInput")
    w_lT_in = nc.dram_tensor("w_lT", [N_LAYERS, HID, HID], dt.float32,
                             kind="ExternalInput")
    b_l_in = nc.dram_tensor("b_l", [N_LAYERS, HID, 1], dt.float32,
                            kind="ExternalInput")
    w_out4_in = nc.dram_tensor("w_out4", [N_LAYERS + 1, HID, OUT_DIM],
                               dt.float16, kind="ExternalInput")
    b_out_in = nc.dram_tensor("b_out", [OUT_DIM, 1], dt.float32,
                              kind="ExternalInput")
    iota_in = nc.dram_tensor("iota_wt", [128, AGW, T], dt.float16,
                             kind="ExternalInput")
    id96_in = nc.dram_tensor("id96", [HID, HID], dt.float16,
                             kind="ExternalInput")
    id128_in = nc.dram_tensor("id128", [128, 128], dt.float32,
                              kind="ExternalInput")
    idx_a_in = nc.dram_tensor("idx_a", [128, NWA * T_a * 8], dt.int16,
                              kind="ExternalInput")
    idx_b_in = nc.dram_tensor("idx_b", [128, NWA * T_b * 8], dt.int16,
                              kind="ExternalInput")
    dstloc_in = nc.dram_tensor("dstloc", [128, NWA, T], dt.float16,
                               kind="ExternalInput")
    out_ext = nc.dram_tensor("out", [NPC, OUT_DIM], dt.float32,
                             kind="ExternalOutput")

    f32, f32r, f16 = dt.float32, dt.float32r, dt.float16

    with tile.TileContext(nc, num_cores=N_CORES) as tc:
        nc.gpsimd.load_library(library_config.mlp)
        with tc.tile_pool(name="persist", bufs=1) as pp, \
             tc.tile_pool(name="xpool", bufs=2) as xpool, \
             tc.tile_pool(name="hp", bufs=3) as hp_pool, \
             tc.tile_pool(name="ga", bufs=3) as ga_pool, \
             tc.tile_pool(name="gb", bufs=3) as gb_pool, \
             tc.tile_pool(name="oha", bufs=5) as oha_pool, \
             tc.tile_pool(name="ohb", bufs=5) as ohb_pool, \
             tc.tile_pool(name="upool", bufs=2) as u_pool, \
             tc.tile_pool(name="otile", bufs=2) as ot_pool, \
             tc.tile_pool(name="ps_agg", bufs=3, space="PSUM") as ps_agg, \
             tc.tile_pool(name="ps_big", bufs=2, space="PSUM") as ps_big, \
             tc.tile_pool(name="ps_tr", bufs=2, space="PSUM") as ps_tr, \
             tc.tile_pool(name="dram", bufs=1, space="DRAM") as dram:

            def load(name, shape, dtype, src_ap):
                t = pp.tile(shape, dtype, name=name)
                nc.sync.dma_start(out=t[:], in_=src_ap)
                return t

            w_inT = load("w_inT", [IN_DIM, HID], f32r, w_inT_in[:].bitcast(f32r))
            wp = load("wp", [IN_DIM, HID], f32r, wp_in[:].bitcast(f32r))
            b_in = load("b_in", [HID, 1], f32, b_in_in[:])
            w_lT = [load(f"w_lT{l}", [HID, HID], f32r, w_lT_in[l].bitcast(f32r))
                    for l in range(N_LAYERS)]
            b_l = [load(f"b_l{l}", [HID, 1], f32, b_l_in[l])
                   for l in range(N_LAYERS)]
            w_out4 = [load(f"w_out4_{s}", [HID, OUT_DIM], f16, w_out4_in[s])
                      for s in range(N_LAYERS + 1)]
            b_out = load("b_out", [OUT_DIM, 1], f32, b_out_in[:])
            iota_wt = load("iota_wt", [128, AGW, T], f16, iota_in[:])
            id96 = load("id96", [HID, HID], f16, id96_in[:])
            id128 = load("id128", [128, 128], f32, id128_in[:])
            idx_a = load("idx_a", [128, NWA * T_a * 8], dt.int16, idx_a_in[:])
            idx_b = load("idx_b", [128, NWA * T_b * 8], dt.int16, idx_b_in[:])
            dstloc = load("dstloc", [128, NWA, T], f16, dstloc_in[:])

            h_state = [pp.tile([HID, NPC], f16, name=f"h{s}")
                       for s in range(N_LAYERS + 1)]
            rm_buf = pp.tile([128, NW, 128], f16, name="rm_buf")
            # zero the pad columns once so gathered rows are NaN-free and
            # the aggregation matmul can use the full 128-col lhsT (FWL)
            nc.vector.memset(rm_buf[:, :, HID:128], 0.0)

            # double-buffered replicated tables: state s lives in buf s%2
            h_fullA = [dram.tile([CLS, 128], f16, name=f"h_fullA{i}",
                                 addr_space="Shared")
                       for i in range(N_LAYERS)]
            h_fullB = [dram.tile([N_NODES - CLS, 128], f16, name=f"h_fullB{i}",
                                 addr_space="Shared")
                       for i in range(N_LAYERS)]
            bounceA = dram.tile([HALF, 128], f16)
            bounceB = dram.tile([NPC - HALF, 128], f16)

            w_chunks = [(c0, min(CHUNK_W, NW - c0))
                        for c0 in range(0, NW, CHUNK_W)]

            def transpose_windows(s, w0, w1):
                for t in range(w0, w1):
                    n0 = t * 128
                    tn = min(128, NPC - n0)
                    pst = ps_tr.tile([128, HID], f16, name="pst")
                    nc.tensor.transpose(pst[:tn, :],
                                        h_state[s][:, n0:n0 + tn], id96[:])
                    nc.scalar.copy(rm_buf[:tn, t, 0:HID], pst[:tn, :])

            def bounce_chunk(j):
                # incremental bounce: ship chunk j's freshly transposed
                # windows to the DRAM staging buffer right away, so the
                # AllGather emitted later waits ~1us instead of a full
                # half-table DMA (AW is CHUNK_W-aligned: chunks 0..7 are
                # exactly the A half)
                c0, cw = w_chunks[j]
                if c0 < AW:
                    nc.sync.dma_start(
                        out=bounceA[c0 * WIN:(c0 + cw) * WIN, :].rearrange(
                            "(t p) d -> p t d", p=128),
                        in_=rm_buf[:, c0:c0 + cw, :])
                    return
                o0 = (c0 - AW) * WIN
                full = cw if c0 + cw < NW else cw - 1
                if full:
                    nc.sync.dma_start(
                        out=bounceB[o0:o0 + full * WIN, :].rearrange(
                            "(t p) d -> p t d", p=128),
                        in_=rm_buf[:, c0:c0 + full, :])
                if c0 + cw == NW:
                    last_n = NPC - (NW - 1) * WIN
                    o1 = o0 + full * WIN
                    nc.sync.dma_start(out=bounceB[o1:o1 + last_n, :],
                                      in_=rm_buf[0:last_n, NW - 1, :])

            def all_gather_a(s):
                nc.gpsimd.collective_compute(
                    "AllGather", mybir.AluOpType.bypass,
                    ins=[bounceA.opt()], outs=[h_fullA[s].opt()],
                    replica_groups=[list(range(N_CORES))])

            def all_gather_b(s):
                nc.gpsimd.collective_compute(
                    "AllGather", mybir.AluOpType.bypass,
                    ins=[bounceB.opt()], outs=[h_fullB[s].opt()],
                    replica_groups=[list(range(N_CORES))])

            qrr = [0]

            def emit_gathers(gbuf, src_view, idx_tile, base_tile, n_tiles):
                for s0 in range(0, n_tiles, GT):
                    sn = min(GT, n_tiles - s0)
                    nc.gpsimd.dma_gather(
                        gbuf[:, s0:s0 + sn, :], src_view,
                        idx_tile[:, (base_tile + s0) * 8:
                                 (base_tile + s0 + sn) * 8],
                        num_idxs=sn * 128, num_idxs_reg=sn * 128,
                        elem_size=128, single_packet=SINGLE_PACKET,
                        queue_num=qrr[0] % 4)
                    qrr[0] += 1

            # ---- GIN layers (layer 0 gathers raw x tables, which are
            # kernel inputs: no startup AllGather, and the h0 projection
            # runs off the critical path inside layer 0's chunk loop) ----
            for l in range(N_LAYERS):
                if l == 0:
                    tblA = xrA_in[:]
                    tblB = xrB_in[:]
                else:
                    tblA = h_fullA[l][:]
                    tblB = h_fullB[l][:]
                ga_t, gb_t = {}, {}

                def emit_A(j, tblA=tblA, ga_t=ga_t):
                    c0a = j * APC
                    cwa = min(APC, NWA - c0a)
                    g = ga_pool.tile([128, APC * T_a, 128], f16,
                                     name="g_a")
                    emit_gathers(g, tblA, idx_a, c0a * T_a, cwa * T_a)
                    ga_t[j] = g

                def emit_B(j, tblB=tblB, gb_t=gb_t):
                    c0a = j * APC
                    cwa = min(APC, NWA - c0a)
                    g = gb_pool.tile([128, APC * T_b, 128], f16,
                                     name="g_b")
                    emit_gathers(g, tblB, idx_b, c0a * T_b, cwa * T_b)
                    gb_t[j] = g

                # The PE stream runs the A-class accumulation of chunk j+2
                # ahead of the B-class accumulation of chunk j: each chunk
                # owns a full PSUM bank whose per-window groups stay open
                # (A part stop=False) until the B tiles close them two
                # slots later.  At a layer boundary the PE therefore has
                # ~2 chunks of A work queued before the first instruction
                # that needs the (still in flight) AllGather-B table.
                ps_t = {}

                def a_chains(j, ps_t=ps_t, ga_t=ga_t):
                    c0a = j * APC
                    cwa = min(APC, NWA - c0a)
                    g_a = ga_t.pop(j)
                    ps = ps_agg.tile([128, CW_N], f32, name="psa")
                    ps_t[j] = ps
                    for wl in range(cwa):
                        w_i = c0a + wl
                        oha = oha_pool.tile([128, AGW, T_a], f16, name="oha")
                        nc.vector.tensor_tensor(
                            oha[:],
                            iota_wt[:, :, 0:T_a],
                            dstloc[:, w_i, 0:T_a].unsqueeze(1)
                                .broadcast_to([128, AGW, T_a]),
                            mybir.AluOpType.is_equal)
                        # full 128-col lhsT (pad cols are zeros) -> FWL
                        # fast weight load; only partitions 0:HID are read.
                        # start=True zeroes the WHOLE 2KB bank (zero-region
                        # granularity), so the bank is one accumulation
                        # group: start only on its very first matmul, stop
                        # only on its very last (in b_chains); unwritten
                        # elements first-write via has_written.
                        for t in range(T_a):
                            nc.tensor.matmul(
                                ps[:, wl * AGW:(wl + 1) * AGW],
                                g_a[:, wl * T_a + t, :], oha[:, :, t],
                                start=(wl == 0 and t == 0), stop=False,
                                skip_group_check=True)

                def b_chains(j, ps_t=ps_t, gb_t=gb_t):
                    c0a = j * APC
                    cwa = min(APC, NWA - c0a)
                    g_b = gb_t.pop(j)
                    ps = ps_t[j]
                    for wl in range(cwa):
                        w_i = c0a + wl
                        ohb = ohb_pool.tile([128, AGW, T_b], f16, name="ohb")
                        nc.vector.tensor_tensor(
                            ohb[:],
                            iota_wt[:, :, 0:T_b],
                            dstloc[:, w_i, T_a:T].unsqueeze(1)
                                .broadcast_to([128, AGW, T_b]),
                            mybir.AluOpType.is_equal)
                        for t in range(T_b):
                            nc.tensor.matmul(
                                ps[:, wl * AGW:(wl + 1) * AGW],
                                g_b[:, wl * T_b + t, :], ohb[:, :, t],
                                start=False,
                                stop=(wl == cwa - 1 and t == T_b - 1),
                                skip_group_check=True)

                emit_A(0)
                emit_A(1)
                emit_A(2)
                emit_B(0)
                emit_B(1)
                a_chains(0)
                a_chains(1)
                for j, (c0, cw) in enumerate(w_chunks):
                    if j + 3 < len(w_chunks):
                        emit_A(j + 3)
                    if j + 2 < len(w_chunks):
                        emit_B(j + 2)
                    b_chains(j)
                    # a_chains(j+2) keeps the PE busy while the DVE runs
                    # chunk j's +h add, so the dense matmul's input is
                    # ready the moment the PE reaches it
                    if j + 2 < len(w_chunks):
                        a_chains(j + 2)
                    n0, cn = c0 * WIN, min(CW_N, NPC - c0 * WIN)
                    ps = ps_t.pop(j)
                    if l == 0:
                        # u = x + sum_e(x_src + c) -> h1 = u @ (W1 Win).T
                        # + (b1 + W1 b_in); the degree*W1@b_in term rides
                        # in via the +c baked into the gathered table rows
                        xb = xpool.tile([IN_DIM, CW_N], f32r, name="xb")
                        nc.sync.dma_start(
                            out=xb[:, :cn],
                            in_=xT_in[:, n0:n0 + cn].bitcast(f32r))
                        u = u_pool.tile([IN_DIM, CW_N], f32r, name="u")
                        nc.vector.tensor_tensor(
                            u[:, :cn], ps[:, :cn], xb[:, :cn],
                            mybir.AluOpType.add)
                        ps2 = ps_big.tile([HID, CW_N], f32, name="psb")
                        nc.tensor.matmul(ps2[:, :cn], wp[:], u[:, :cn],
                                         start=True, stop=True)
                        nc.scalar.add(h_state[1][:, n0:n0 + cn],
                                      ps2[:, :cn], b_l[0][:])
                        # h0 projection, needed only by the final output
                        # concat: off the critical path
                        psh = ps_big.tile([HID, CW_N], f32, name="psh",
                                          tag="psb")
                        nc.tensor.matmul(psh[:, :cn], w_inT[:], xb[:, :cn],
                                         start=True, stop=True)
                        nc.scalar.add(h_state[0][:, n0:n0 + cn],
                                      psh[:, :cn], b_in[:])
                    else:
                        hp = hp_pool.tile([HID, CW_N], f32r, name="hp")
                        nc.vector.tensor_tensor(
                            hp[:, :cn], ps[:HID, :cn],
                            h_state[l][:, n0:n0 + cn], mybir.AluOpType.add)
                        ps2 = ps_big.tile([HID, CW_N], f32, name="psb")
                        nc.tensor.matmul(ps2[:, :cn], w_lT[l][:], hp[:, :cn],
                                         start=True, stop=True)
                        nc.scalar.add(h_state[l + 1][:, n0:n0 + cn],
                                      ps2[:, :cn], b_l[l][:])
                    if l < N_LAYERS - 1:
                        transpose_windows(l + 1, c0, c0 + cw)
                        bounce_chunk(j)
                        if j == BOUNCE_A_CHUNK:
                            all_gather_a(l + 1)
                    else:
                        # interleave the output projection into layer 2
                        pso = ps_big.tile([OUT_DIM, CW_N], f32, name="pso",
                                          tag="psb")
                        for s in range(N_LAYERS + 1):
                            nc.tensor.matmul(pso[:, :cn], w_out4[s][:],
                                             h_state[s][:, n0:n0 + cn],
                                             start=(s == 0),
                                             stop=(s == N_LAYERS))
                        ot = ot_pool.tile([OUT_DIM, CW_N], f32, name="ot")
                        nc.scalar.add(ot[:, :cn], pso[:, :cn], b_out[:])
                        for tt in range(-(-cn // 128)):
                            t0 = tt * 128
                            tn = min(128, cn - t0)
                            pst = ps_tr.tile([128, 128], f32, name="psto",
                                             tag="pst")
                            nc.tensor.transpose(pst[:tn, :],
                                                ot[:, t0:t0 + tn], id128[:])
                            orow = ot_pool.tile([128, 128], f32, name="orow")
                            nc.scalar.copy(orow[:tn, :], pst[:tn, :])
                            nc.sync.dma_start(
                                out=out_ext[n0 + t0:n0 + t0 + tn, :],
                                in_=orow[:tn, :])
                if l < N_LAYERS - 1:
                    all_gather_b(l + 1)

    nc.compile()
    return nc


def _get_nc_and_inputs(inputs):
    from concourse import bass_utils  # noqa: F401  (path setup)

    x = np.asarray(inputs["x"], np.float32)
    edge_index = np.asarray(inputs["edge_index"], np.int32)
    W_in = np.asarray(inputs["W_in"], np.float32)
    b_in = np.asarray(inputs["b_in"], np.float32)
    W_layers = np.asarray(inputs["W_layers"], np.float32)
    b_layers = np.asarray(inputs["b_layers"], np.float32)
    W_out = np.asarray(inputs["W_out"], np.float32)
    b_out = np.asarray(inputs["b_out"], np.float32)

    idx_a_w, idx_b_w, dstloc, T_a, T_b, perm = _prep(edge_index)

    key = ("nc", T_a, T_b)
    if key not in _cache:
        _cache.clear()
        _cache[key] = _build(T_a, T_b)
    nc = _cache[key]

    T = T_a + T_b
    inv = np.empty(N_NODES, np.int64)
    inv[perm] = np.arange(N_NODES)
    xT = np.ascontiguousarray(x.T[:, inv])
    w_inT = np.ascontiguousarray(W_in.T)
    w_lT = np.ascontiguousarray(W_layers.transpose(0, 2, 1))
    b_l = np.ascontiguousarray(b_layers[:, :, None]).copy()

    # Layer-0 reformulation: h1 = (x + A x + deg*c) @ (W1 Win).T
    #   + (b1 + W1 b_in), with W' c = W1 b_in solved exactly so the
    # per-node degree bias rides in through +c on every gathered x row.
    W1 = W_layers[0]
    Wp = (W1 @ W_in).astype(np.float64)          # [HID, IN_DIM]
    bpp = (W1 @ b_in).astype(np.float64)         # [HID]
    c_vec = Wp.T @ np.linalg.solve(Wp @ Wp.T, bpp)
    wp_t = np.ascontiguousarray(Wp.T.astype(np.float32))   # [IN_DIM, HID]
    b_l[0, :, 0] = b_layers[0] + bpp.astype(np.float32)
    xr = x[inv]                                   # x rows under new ids
    xr_tbl = (xr + c_vec[None, :]).astype(np.float16)
    xrA = np.concatenate(
        [xr_tbl[c * NPC:c * NPC + HALF] for c in range(N_CORES)], 0)
    xrB = np.concatenate(
        [xr_tbl[c * NPC + HALF:(c + 1) * NPC] for c in range(N_CORES)], 0)
    xrA = np.ascontiguousarray(xrA)
    xrB = np.ascontiguousarray(xrB)
    w_out4 = np.ascontiguousarray(
        np.stack([W_out[:, s * HID:(s + 1) * HID].T
                  for s in range(N_LAYERS + 1)])).astype(np.float16)
    iota_wt = np.ascontiguousarray(np.broadcast_to(
        np.arange(AGW, dtype=np.float16)[None, :, None],
        (128, AGW, T)))
    id96 = np.eye(HID, dtype=np.float16)
    id128 = np.eye(128, dtype=np.float32)

    in_maps = []
    for c in range(N_CORES):
        in_maps.append({
            "xT": np.ascontiguousarray(xT[:, c * NPC:(c + 1) * NPC]),
            "xrA": xrA,
            "xrB": xrB,
            "wp": wp_t,
            "w_inT": w_inT,
            "b_in": b_in.reshape(HID, 1),
            "w_lT": w_lT,
            "b_l": b_l,
            "w_out4": w_out4,
            "b_out": b_out.reshape(OUT_DIM, 1),
            "iota_wt": iota_wt,
            "id96": id96,
            "id128": id128,
            "idx_a": idx_a_w[c],
            "idx_b": idx_b_w[c],
            "dstloc": dstloc[c],
        })
    return nc, in_maps, perm


def run(inputs, trace=False):
    from concourse import bass_utils

    nc, in_maps, perm = _get_nc_and_inputs(inputs)
    res = bass_utils.run_bass_kernel_spmd(
        nc, in_maps, core_ids=list(range(N_CORES)), trace=trace)
    out = np.concatenate([res.results[c]["out"] for c in range(N_CORES)], 0)
    return out[perm], res


def kernel(**inputs):
    out, _ = run(inputs, trace=False)
    return out

